# revision 1
# baseline (speedup 1.0000x reference)
"""Self-contained TRN2 Bass kernel for axial attention (nn_AxialAttention).

kernel(**inputs) takes FULL inputs (x [8,128,128,512], Wq/Wk/Wv/Wo [512,512],
bo [512]) and returns the FULL output [8,128,128,512] (float32).

Strategy: data-parallel over N across 8 NeuronCores (core c computes image c).
Per core: fp32r projections/output matmul, bf16 attention middle, softmax
without max-subtraction (logits ~N(0,1)), PE transposes for X^T and A^T,
engine-balanced copies (ACT=exp only, DVE=psum exits, per-head accum sums).
"""
import sys
sys.path.insert(0, "/opt/trn_rl_repo")
sys.path.insert(0, "/root/.axon_site/_ro/trn_rl_repo")

"""Axial attention Bass kernel for TRN2 — builder shared by test.py and kernel.py.

Problem: x [N=8, H=128, W=128, C=512], attention along H (8 heads, head dim 64):
  per (n, w): seq = x[n, :, w, :] [128, 512]
  q/k/v = seq @ W{q,k,v}.T ; per head S = q k^T/8 ; A = softmax_j(S) ; out = A v
  O = out @ Wo.T + bo  -> out[n, :, w, :]

Sharding: data-parallel over N — core c computes image n=c entirely.

Per-core layout strategy (tokens t = h, seqs s = w, 4 seqs per block):
  X_blk  [128 t, 4 s, 512 c]   <- DMA (f32r)
  X^T    [128 c_loc, 4 jc, 4 s, 128 t]  via PE transpose (f32r)
  Q^T/K^T [128 co_loc, 4 co, 4 s, 128 t] = W^T.T @ X^T  (f32r matmul, bf16 out)
  V      [128 t, 4 s, 512 c]  (bf16 out)
  S_g    [128 i, 128 j] psum = Q^T_g.T @ K^T_g  (bf16)
  A      exp(S/8) -> [128 i, 8 g, 128 j] bf16 + per-g rowsum (ACT accum)
  AN     A * (1/rowsum) broadcast  (DVE)
  A^T    per-head DMA xbar transpose -> [128 j, 8 g, 128 i] bf16
  outT   [128 c_loc, 4 jc, 128 i] psum = V_g.T...: lhsT=V_g, rhs=A^T_g
  O      [128 t, 512 co] = outT.T @ Wo^T (f32r) + bo
"""
import numpy as np

import concourse.bass as bass
import concourse.bacc as bacc
import concourse.tile as tile
from concourse import mybir

F32 = mybir.dt.float32
F32R = mybir.dt.float32r
BF16 = mybir.dt.bfloat16
EXP = mybir.ActivationFunctionType.Exp

H = 128   # tokens per sequence (attention axis)
W = 128   # sequences per core
C = 512
G = 8     # heads
GP = C // G  # 64
BLK = 4   # sequences per block
NBLK = W // BLK
NCHUNK = C // 128  # 4 k-chunks


def build_kernel(num_cores=8, attn_f32=False, w_total=W, reps=1, an_engine='dve', at_engine='dve', psum_bufs=(3,2,3), exp_mode='perhead', attn_mode='std', sbufs=None, psum_unified=False):
    """Build + compile the Bass module. Returns nc.

    reps>1 wraps the whole computation in a dynamic loop (for timing by
    wall-clock differencing; results are identical, just recomputed)."""
    nblk = w_total // BLK
    nc = bacc.Bacc("TRN2", target_bir_lowering=False, debug=False,
                   num_devices=num_cores)

    x_d = nc.dram_tensor("x", [H, w_total, C], F32R, kind="ExternalInput").ap()
    wq_d = nc.dram_tensor("wqT", [C, C], F32R, kind="ExternalInput").ap()
    wk_d = nc.dram_tensor("wkT", [C, C], F32R, kind="ExternalInput").ap()
    wv_d = nc.dram_tensor("wvT", [C, C], F32R, kind="ExternalInput").ap()
    wo_d = nc.dram_tensor("woT", [C, C], F32R, kind="ExternalInput").ap()
    bo_d = nc.dram_tensor("bo", [C], F32, kind="ExternalInput").ap()
    id_d = nc.dram_tensor("ident", [128, 128], F32R, kind="ExternalInput").ap()
    out_d = nc.dram_tensor("out", [H, w_total, C], F32, kind="ExternalOutput").ap()

    AMID = F32R if attn_f32 else BF16  # dtype of attention middle section

    sb_bufs = sbufs or {}
    def B(name, d):
        return sb_bufs.get(name, d)
    with tile.TileContext(nc) as tc:
        with tc.tile_pool(name="consts", bufs=1) as consts, \
             tc.tile_pool(name="px", bufs=B('px', 2)) as px, \
             tc.tile_pool(name="pxt", bufs=B('pxt', 2)) as pxt, \
             tc.tile_pool(name="pqt", bufs=B('pqt', 2)) as pqt, \
             tc.tile_pool(name="pv", bufs=B('pv', 2)) as pv, \
             tc.tile_pool(name="pa", bufs=B('pa', 3)) as pa, \
             tc.tile_pool(name="pstat", bufs=B('pstat', 4)) as pstat, \
             tc.tile_pool(name="pot", bufs=B('pot', 2)) as pot, \
             tc.tile_pool(name="po", bufs=B('po', 2)) as po, \
             tc.tile_pool(name="pdram", bufs=6, space="DRAM") as pdram, \
             tc.tile_pool(name="psf", bufs=(8 if psum_unified else psum_bufs[0]), space="PSUM") as psf, \
             tc.tile_pool(name="pss", bufs=psum_bufs[1], space="PSUM") as _pss, \
             tc.tile_pool(name="psb", bufs=psum_bufs[2], space="PSUM") as _psb:
            if psum_unified:
                class _U:
                    _n = [0]
                    def tile(self, shape, dtype, tag=None):
                        self._n[0] += 1
                        return psf.tile(shape, dtype, tag="f",
                                        name=f"u{self._n[0]}")
                pss = psb = _U()
            else:
                pss, psb = _pss, _psb

            # ---- constants ----
            wq_sb = consts.tile([128, NCHUNK, C], F32R, tag="wq")
            wk_sb = consts.tile([128, NCHUNK, C], F32R, tag="wk")
            wv_sb = consts.tile([128, NCHUNK, C], F32R, tag="wv")
            wo_sb = consts.tile([128, NCHUNK, C], F32R, tag="wo")
            for w_sb, w_d in ((wq_sb, wq_d), (wk_sb, wk_d), (wv_sb, wv_d),
                              (wo_sb, wo_d)):
                nc.sync.dma_start(w_sb[:], w_d.rearrange("(j p) c -> p j c", p=128))
            bo_sb = consts.tile([128, C], F32, tag="bo")
            nc.sync.dma_start(
                bo_sb[:],
                bo_d.rearrange("(o c) -> o c", o=1).broadcast_to((128, C)))
            id_sb = consts.tile([128, 128], F32R, tag="id")
            nc.sync.dma_start(id_sb[:], id_d[:])
            id_bf = consts.tile([128, 128], BF16, tag="idbf")
            nc.vector.tensor_copy(id_bf[:], id_sb[:].bitcast(F32))
            ones_bf = consts.tile([128, 128], BF16, tag="ones")
            nc.vector.memset(ones_bf[:], 1.0)

            state = {}

            def front(b):
                X_blk = px.tile([128, BLK, C], F32R, tag="x")
                nc.sync.dma_start(X_blk[:], x_d[:, b * BLK:(b + 1) * BLK, :])
                XT_sb = pxt.tile([128, NCHUNK, BLK, 128], F32R, tag="xt")
                for s in range(BLK):
                    XT_ps = psf.tile([128, NCHUNK, 128], F32R, tag="f")
                    for jc in range(NCHUNK):
                        nc.tensor.transpose(
                            XT_ps[:, jc, :],
                            X_blk[:, s, jc * 128:(jc + 1) * 128], id_sb[:])
                    nc.vector.tensor_copy(XT_sb[:, :, s, :], XT_ps[:])
                QT = pqt.tile([128, NCHUNK, BLK, 128], AMID, tag="qt")
                KT = pqt.tile([128, NCHUNK, BLK, 128], AMID, tag="kt")
                for w_sb, dst in ((wq_sb, QT), (wk_sb, KT)):
                    for co in range(NCHUNK):
                        PT = psf.tile([128, BLK * 128], F32, tag="f")
                        for jc in range(NCHUNK):
                            nc.tensor.matmul(
                                PT[:],
                                lhsT=w_sb[:, jc, co * 128:(co + 1) * 128],
                                rhs=XT_sb[:, jc, :, :],
                                start=(jc == 0), stop=(jc == NCHUNK - 1))
                        nc.vector.tensor_copy(dst[:, co, :, :], PT[:])
                V = pv.tile([128, BLK, C], AMID, tag="v")
                for s in range(BLK):
                    VP = psf.tile([128, C], F32, tag="f")
                    for jc in range(NCHUNK):
                        nc.tensor.matmul(
                            VP[:], lhsT=XT_sb[:, jc, s, :],
                            rhs=wv_sb[:, jc, :],
                            start=(jc == 0), stop=(jc == NCHUNK - 1))
                    nc.vector.tensor_copy(V[:, s, :], VP[:])
                state[b] = (QT, KT, V)

            def back(b):
                QT, KT, V = state.pop(b)
                O_sb = po.tile([128, BLK, C], F32, tag="o")
                for s in range(BLK):
                    A = pa.tile([128, G, 128], AMID, tag="a")
                    sums = pstat.tile([128, G], F32, tag="sums")
                    # Even heads (PE row-group 0) and odd heads (row-group 1)
                    # run concurrently in the array -> MUST land in different
                    # PSUM banks (same-bank concurrent row-group writes hang).
                    S_e = pss.tile([128, G // 2, 128], F32, tag="s")
                    S_o = pss.tile([128, G // 2, 128], F32, tag="s")
                    for g in range(G):
                        p0 = 64 * (g % 2)
                        S_ps = S_e if g % 2 == 0 else S_o
                        nc.tensor.matmul(
                            S_ps[:, g // 2, :],
                            lhsT=QT[p0:p0 + 64, g // 2, s, :],
                            rhs=KT[p0:p0 + 64, g // 2, s, :],
                            start=True, stop=True)
                    if exp_mode == 'perhead':
                        for g in range(G):
                            S_ps = S_e if g % 2 == 0 else S_o
                            nc.scalar.activation(
                                A[:, g, :], S_ps[:, g // 2, :], EXP,
                                scale=1.0 / np.sqrt(GP),
                                accum_out=sums[:, g:g + 1])
                    else:
                        # batched: 2 exps; A slices by even/odd heads; sums via
                        # DVE reduce over [128, 4, 128] per half
                        a_ev = A[:, 0:G:2, :]
                        a_od = A[:, 1:G:2, :]
                        nc.scalar.activation(a_ev, S_e[:], EXP,
                                             scale=1.0 / np.sqrt(GP))
                        nc.scalar.activation(a_od, S_o[:], EXP,
                                             scale=1.0 / np.sqrt(GP))
                        se = sums[:, 0:G:2].rearrange("p (g o) -> p g o", o=1)
                        so = sums[:, 1:G:2].rearrange("p (g o) -> p g o", o=1)
                        nc.vector.reduce_sum(se, a_ev, axis=mybir.AxisListType.X)
                        nc.vector.reduce_sum(so, a_od, axis=mybir.AxisListType.X)
                    rcp = pstat.tile([128, G], F32, tag="rcp")
                    nc.vector.reciprocal(rcp[:], sums[:])
                    AN = pa.tile([128, G, 128], AMID, tag="an")
                    rcp_b = rcp[:].rearrange("p (o g) -> p o g", o=1) \
                        .rearrange("p o g -> p g o") \
                        .broadcast_to((128, G, 128))
                    if an_engine == 'pool':
                        nc.gpsimd.tensor_mul(AN[:], A[:], rcp_b)
                    else:
                        nc.vector.tensor_mul(AN[:], A[:], rcp_b)
                    # A^T via PE transposes (xbar DMA transpose races with
                    # concurrent DRAM DMA traffic on this HW/runtime).
                    AT = pa.tile([128, G, 128], AMID, tag="at")
                    idt = id_bf if AMID == BF16 else id_sb
                    for half in range(2):
                        ATp = psb.tile([128, G // 2, 128], AMID, tag="b")
                        for gg in range(G // 2):
                            g = half * (G // 2) + gg
                            nc.tensor.transpose(ATp[:, gg, :], AN[:, g, :],
                                                idt[:])
                        eng = nc.vector if at_engine == 'dve' else nc.scalar
                        if at_engine == 'dve':
                            nc.vector.tensor_copy(
                                AT[:, half * (G // 2):(half + 1) * (G // 2), :],
                                ATp[:])
                        else:
                            nc.scalar.copy(
                                AT[:, half * (G // 2):(half + 1) * (G // 2), :],
                                ATp[:])
                    OT_ps = psb.tile([128, NCHUNK, 128], F32, tag="b")
                    for g in range(G):
                        p0 = 64 * (g % 2)
                        nc.tensor.matmul(
                            OT_ps[p0:p0 + 64, g // 2, :],
                            lhsT=V[:, s, 64 * g:64 * (g + 1)],
                            rhs=AT[:, g, :],
                            start=True, stop=True)
                    OT_sb = pot.tile([128, NCHUNK, 128], F32R, tag="ot")
                    nc.vector.tensor_copy(OT_sb[:], OT_ps[:])
                    O_ps = psb.tile([128, C], F32, tag="b")
                    for jc in range(NCHUNK):
                        nc.tensor.matmul(
                            O_ps[:], lhsT=OT_sb[:, jc, :],
                            rhs=wo_sb[:, jc, :],
                            start=(jc == 0), stop=(jc == NCHUNK - 1))
                    nc.vector.tensor_add(O_sb[:, s, :], O_ps[:], bo_sb[:])
                nc.sync.dma_start(out_d[:, b * BLK:(b + 1) * BLK, :], O_sb[:])

            def back_st(b):
                # S^T-direct attention: exp(S^T) IS A^T (no transposes, no
                # normalization multiply on A). Softmax sums via GpSimd
                # cross-partition reduce; normalization folded into the
                # OT psum->sbuf copy using a reciprocal tile broadcast
                # through a DRAM scratch.
                QT, KT, V = state.pop(b)
                O_sb = po.tile([128, BLK, C], F32, tag="o")
                for s in range(BLK):
                    AT = pa.tile([128, G, 128], AMID, tag="at")
                    S_e = pss.tile([128, G // 2, 128], F32, tag="s")
                    S_o = pss.tile([128, G // 2, 128], F32, tag="s")
                    for g in range(G):
                        p0 = 64 * (g % 2)
                        S_ps = S_e if g % 2 == 0 else S_o
                        # lhsT=K^T, rhs=Q^T  ->  S^T[j, i]
                        nc.tensor.matmul(
                            S_ps[:, g // 2, :],
                            lhsT=KT[p0:p0 + 64, g // 2, s, :],
                            rhs=QT[p0:p0 + 64, g // 2, s, :],
                            start=True, stop=True)
                    if exp_mode == 'perhead':
                        for g in range(G):
                            S_ps = S_e if g % 2 == 0 else S_o
                            nc.scalar.activation(
                                AT[:, g, :], S_ps[:, g // 2, :], EXP,
                                scale=1.0 / np.sqrt(GP))
                    else:
                        nc.scalar.activation(AT[:, 0:G:2, :], S_e[:], EXP,
                                             scale=1.0 / np.sqrt(GP))
                        nc.scalar.activation(AT[:, 1:G:2, :], S_o[:], EXP,
                                             scale=1.0 / np.sqrt(GP))
                    # sums[g, i] = sum_j A^T[j, g, i]  (partition reduce)
                    sums = pstat.tile([1, G, 128], F32, tag="sums")
                    nc.gpsimd.reduce_sum(sums[:], AT[:],
                                         axis=mybir.AxisListType.C)
                    # broadcast sums to [128, jc, i]: row (2jc + p//64)
                    sc_d = pdram.tile([1, G * 128], F32, tag="scr")
                    nc.sync.dma_start(sc_d[:], sums[:])
                    st = pstat.tile([128, NCHUNK, 128], F32, tag="st")
                    s4 = sc_d[:].rearrange("o (jc hg i) -> (o hg) jc i",
                                           hg=2, i=128)
                    for hg in range(2):
                        src = s4[hg:hg + 1].broadcast_to((64, NCHUNK, 128))
                        nc.sync.dma_start(st[hg * 64:(hg + 1) * 64, :, :], src)
                    rcp_t = pstat.tile([128, NCHUNK, 128], F32, tag="rcpt")
                    nc.vector.reciprocal(rcp_t[:], st[:])
                    OT_ps = psb.tile([128, NCHUNK, 128], F32, tag="b")
                    for g in range(G):
                        p0 = 64 * (g % 2)
                        nc.tensor.matmul(
                            OT_ps[p0:p0 + 64, g // 2, :],
                            lhsT=V[:, s, 64 * g:64 * (g + 1)],
                            rhs=AT[:, g, :],
                            start=True, stop=True)
                    OT_sb = pot.tile([128, NCHUNK, 128], F32R, tag="ot")
                    nc.vector.tensor_mul(OT_sb[:], OT_ps[:], rcp_t[:])
                    O_ps = psb.tile([128, C], F32, tag="b")
                    for jc in range(NCHUNK):
                        nc.tensor.matmul(
                            O_ps[:], lhsT=OT_sb[:, jc, :],
                            rhs=wo_sb[:, jc, :],
                            start=(jc == 0), stop=(jc == NCHUNK - 1))
                    nc.vector.tensor_add(O_sb[:, s, :], O_ps[:], bo_sb[:])
                nc.sync.dma_start(out_d[:, b * BLK:(b + 1) * BLK, :], O_sb[:])

            def back_st2(b):
                # S^T-direct: exp(S^T) IS A^T. Softmax sums via all-ones
                # matmul (broadcasts column sums to every partition in PSUM);
                # normalization folded into the OT psum->sbuf copy.
                QT, KT, V = state.pop(b)
                O_sb = po.tile([128, BLK, C], F32, tag="o")
                for s in range(BLK):
                    AT = pa.tile([128, G, 128], AMID, tag="at")
                    S_e = pss.tile([128, G // 2, 128], F32, tag="s")
                    S_o = pss.tile([128, G // 2, 128], F32, tag="s")
                    for g in range(G):
                        p0 = 64 * (g % 2)
                        S_ps = S_e if g % 2 == 0 else S_o
                        nc.tensor.matmul(
                            S_ps[:, g // 2, :],
                            lhsT=KT[p0:p0 + 64, g // 2, s, :],
                            rhs=QT[p0:p0 + 64, g // 2, s, :],
                            start=True, stop=True)
                    if exp_mode == 'perhead':
                        for g in range(G):
                            S_ps = S_e if g % 2 == 0 else S_o
                            nc.scalar.activation(
                                AT[:, g, :], S_ps[:, g // 2, :], EXP,
                                scale=1.0 / np.sqrt(GP))
                    else:
                        nc.scalar.activation(AT[:, 0:G:2, :], S_e[:], EXP,
                                             scale=1.0 / np.sqrt(GP))
                        nc.scalar.activation(AT[:, 1:G:2, :], S_o[:], EXP,
                                             scale=1.0 / np.sqrt(GP))
                    # column sums broadcast to all partitions:
                    # bc_e[p, g*128+i] = sum_j AT[j, g, i]  (heads 0-3)
                    bc_e = psb.tile([128, 512], F32, tag="b")
                    bc_o = psb.tile([128, 512], F32, tag="b")
                    nc.tensor.matmul(bc_e[:], lhsT=ones_bf[:],
                                     rhs=AT[:, 0:4, :], start=True, stop=True)
                    nc.tensor.matmul(bc_o[:], lhsT=ones_bf[:],
                                     rhs=AT[:, 4:8, :], start=True, stop=True)
                    # rcp_t[p, jc, i] = 1/sums[2jc + p//64, i]
                    rcp_t = pstat.tile([128, NCHUNK, 128], F32, tag="rcpt")
                    for hg in range(2):
                        pr = slice(hg * 64, (hg + 1) * 64)
                        for hc, bc in ((0, bc_e), (1, bc_o)):
                            # [64, 2, 128] view: cols hg*128 + jc*256
                            src = bc[pr, :].rearrange(
                                "p (jc r i) -> p jc r i", jc=2, i=128)[
                                :, :, hg, :]
                            nc.vector.reciprocal(
                                rcp_t[pr, 2 * hc:2 * hc + 2, :], src)
                    OT_ps = psb.tile([128, NCHUNK, 128], F32, tag="b")
                    for g in range(G):
                        p0 = 64 * (g % 2)
                        nc.tensor.matmul(
                            OT_ps[p0:p0 + 64, g // 2, :],
                            lhsT=V[:, s, 64 * g:64 * (g + 1)],
                            rhs=AT[:, g, :],
                            start=True, stop=True)
                    OT_sb = pot.tile([128, NCHUNK, 128], F32R, tag="ot")
                    nc.vector.tensor_mul(OT_sb[:], OT_ps[:], rcp_t[:])
                    O_ps = psb.tile([128, C], F32, tag="b")
                    for jc in range(NCHUNK):
                        nc.tensor.matmul(
                            O_ps[:], lhsT=OT_sb[:, jc, :],
                            rhs=wo_sb[:, jc, :],
                            start=(jc == 0), stop=(jc == NCHUNK - 1))
                    nc.vector.tensor_add(O_sb[:, s, :], O_ps[:], bo_sb[:])
                nc.sync.dma_start(out_d[:, b * BLK:(b + 1) * BLK, :], O_sb[:])

            back_fn = {'st': back_st, 'st2': back_st2}.get(attn_mode, back)

            def whole():
                for b in range(nblk + 1):
                    if b < nblk:
                        front(b)
                    if b >= 1:
                        back_fn(b - 1)

            if reps == 1:
                whole()
            else:
                with tc.For_i(0, reps, 1):
                    whole()

    nc.compile()
    return nc


def make_in_maps(x, Wq, Wk, Wv, Wo, bo, num_cores=8):
    """Full inputs -> per-core input dicts (data-parallel over N)."""
    x = np.asarray(x, dtype=np.float32)
    ident = np.eye(128, dtype=np.float32)
    wqT = np.ascontiguousarray(np.asarray(Wq, np.float32).T)
    wkT = np.ascontiguousarray(np.asarray(Wk, np.float32).T)
    wvT = np.ascontiguousarray(np.asarray(Wv, np.float32).T)
    woT = np.ascontiguousarray(np.asarray(Wo, np.float32).T)
    bo = np.asarray(bo, np.float32)
    return [{"x": np.ascontiguousarray(x[n]), "wqT": wqT, "wkT": wkT,
             "wvT": wvT, "woT": woT, "bo": bo, "ident": ident}
            for n in range(num_cores)]


_NC_CACHE = {}


def kernel(x, Wq, Wk, Wv, Wo, bo):
    import numpy as np
    from concourse import bass_utils

    if "nc" not in _NC_CACHE:
        _NC_CACHE["nc"] = build_kernel(
            num_cores=8, attn_mode="std", psum_bufs=(2, 3, 3),
            sbufs={"px": 3, "pxt": 3, "pqt": 3, "pv": 3, "pa": 4,
                   "pstat": 6, "pot": 3, "po": 3})
    nc = _NC_CACHE["nc"]
    in_maps = make_in_maps(x, Wq, Wk, Wv, Wo, bo, num_cores=8)
    res = bass_utils.run_bass_kernel_spmd(nc, in_maps, core_ids=list(range(8)))
    out = np.stack([res.results[c]["out"] for c in range(8)], axis=0)
    return out.astype(np.float32)



# revision 19
# speedup vs baseline: 4.9922x; 4.9922x over previous
"""Self-contained TRN2 Bass kernel for axial attention (nn_AxialAttention).

kernel(**inputs) takes FULL inputs (x [8,128,128,512], Wq/Wk/Wv/Wo [512,512],
bo [512]) and returns the FULL output [8,128,128,512] (float32).

Sharding: data-parallel over N across 8 NeuronCores (core c computes image c).

Per-core dataflow (v2), per block of BLK=4 sequences (seq = one W column,
tokens = H axis):
  X_blk  [128 t, 4 s, 512 c] f32  <- DMA
  X^T    [128 c_loc, 4 jc, 4 s, 128 t] bf16, PE transposes (f32r in, bf16 out)
  Q^T/K^T [128 co_loc, 4 co, 4 s, 128 t] bf16 = Wq/k^T.T @ X^T, psum exit on ACT
  V      [128 t, 4 s, 512 c] bf16 = X^T.T @ Wv^T, psum exit on ACT
  S^T_g  [128 j, 128 i] psum = K^T_g.T @ Q^T_g   (even/odd heads -> 2 banks)
  A^T    exp(S^T/8) directly (no transposes; softmax max-subtraction skipped:
         logits ~N(0,1), |S|<~6, exp safe in f32)
  colsum broadcast: ones^T @ A^T (per parity) -> bc psum [128, 4, 128]
  rcp_t  1/bc on the used partition halves (DVE, 2 ops)
  OT     [128 c_loc, 4 jc, 128 i] psum = V_g.T...: lhsT=V_g, rhs=A^T_g
  OT_sb  = OT * rcp_t  (softmax normalization folded into psum exit, DVE)
  O      [128 t, 512 co] = OT.T @ Wo^T (f32r) + bo (DVE psum exit)
"""
import sys
sys.path.insert(0, "/opt/trn_rl_repo")
sys.path.insert(0, "/root/.axon_site/_ro/trn_rl_repo")

import numpy as np

import concourse.bass as bass
import concourse.bacc as bacc
import concourse.tile as tile
from concourse import mybir
from concourse import bass_isa

F32 = mybir.dt.float32
F32R = mybir.dt.float32r
BF16 = mybir.dt.bfloat16
EXP = mybir.ActivationFunctionType.Exp

H = 128   # tokens per sequence (attention axis)
W = 128   # sequences per core
C = 512
G = 8     # heads
GP = C // G  # 64
BLK = 4   # sequences per block
NCHUNK = C // 128  # 4 k-chunks


def build_kernel(num_cores=8, attn_f32=False, w_total=W, reps=1,
                 qk_copy='act', v_copy='act', xt_copy='dve',
                 ot_mul='dve', xconv='pool', psum_bufs=(3, 2, 3), sbufs=None,
                 psum_unified=False, pre_x0=True, sums='bc'):
    """Build + compile the Bass module. Returns nc.

    reps>1 wraps the computation in a dynamic loop (for wall-clock
    rep-differencing timing; results identical, just recomputed)."""
    nblk = w_total // BLK
    nc = bacc.Bacc("TRN2", target_bir_lowering=False, debug=False,
                   num_devices=num_cores)

    x_d = nc.dram_tensor("x", [H, w_total, C], F32R, kind="ExternalInput").ap()
    wq_d = nc.dram_tensor("wqT", [C, C], F32R, kind="ExternalInput").ap()
    wk_d = nc.dram_tensor("wkT", [C, C], F32R, kind="ExternalInput").ap()
    wv_d = nc.dram_tensor("wvT", [C, C], F32R, kind="ExternalInput").ap()
    wo_d = nc.dram_tensor("woT", [C, C], F32R, kind="ExternalInput").ap()
    bo_d = nc.dram_tensor("bo", [C], F32, kind="ExternalInput").ap()
    id_d = nc.dram_tensor("ident", [128, 128], F32R, kind="ExternalInput").ap()
    out_d = nc.dram_tensor("out", [H, w_total, C], F32, kind="ExternalOutput").ap()

    AMID = F32R if attn_f32 else BF16

    sb_bufs = sbufs or {}
    def B(name, d):
        return sb_bufs.get(name, d)

    def cp_engine(which):
        return nc.scalar.copy if which == 'act' else nc.vector.tensor_copy

    with tile.TileContext(nc) as tc:
        with tc.tile_pool(name="consts", bufs=1) as consts, \
             tc.tile_pool(name="px", bufs=B('px', 3)) as px, \
             tc.tile_pool(name="pxt", bufs=B('pxt', 3)) as pxt, \
             tc.tile_pool(name="pqt", bufs=B('pqt', 3)) as pqt, \
             tc.tile_pool(name="pv", bufs=B('pv', 3)) as pv, \
             tc.tile_pool(name="pa", bufs=B('pa', 4)) as pa, \
             tc.tile_pool(name="pstat", bufs=B('pstat', 4)) as pstat, \
             tc.tile_pool(name="pot", bufs=B('pot', 3)) as pot, \
             tc.tile_pool(name="po", bufs=B('po', 3)) as po, \
             tc.tile_pool(name="psf", bufs=(8 if psum_unified else psum_bufs[0]),
                          space="PSUM") as psf, \
             tc.tile_pool(name="pss", bufs=psum_bufs[1], space="PSUM") as _pss, \
             tc.tile_pool(name="psb", bufs=psum_bufs[2], space="PSUM") as _psb:
            if psum_unified:
                class _U:
                    _n = [0]
                    def tile(self, shape, dtype, tag=None):
                        self._n[0] += 1
                        return psf.tile(shape, dtype, tag="f",
                                        name=f"u{self._n[0]}")
                pss = psb = _U()
            else:
                pss, psb = _pss, _psb

            # ---- block-0 input DMA first: the first PE work (transposes of
            # X0) must not wait behind 4MB of weight DMAs in the queue ----
            X0 = None
            if pre_x0 and reps == 1:
                X0 = px.tile([128, BLK, C], F32R, tag="x")
                nc.sync.dma_start(X0[:], x_d[:, 0:BLK, :])

            # ---- constants (id first: the first PE work -- transposes of X0
            # -- needs only X0 + id, not the 4MB of weights) ----
            id_sb = consts.tile([128, 128], F32R, tag="id")
            nc.sync.dma_start(id_sb[:], id_d[:])
            wq_sb = consts.tile([128, NCHUNK, C], F32R, tag="wq")
            wk_sb = consts.tile([128, NCHUNK, C], F32R, tag="wk")
            wv_sb = consts.tile([128, NCHUNK, C], F32R, tag="wv")
            wo_sb = consts.tile([128, NCHUNK, C], F32R, tag="wo")
            for w_sb, w_d in ((wq_sb, wq_d), (wk_sb, wk_d), (wv_sb, wv_d),
                              (wo_sb, wo_d)):
                nc.sync.dma_start(w_sb[:], w_d.rearrange("(j p) c -> p j c", p=128))
            bo_sb = consts.tile([128, C], F32, tag="bo")
            nc.sync.dma_start(
                bo_sb[:],
                bo_d.rearrange("(o c) -> o c", o=1).broadcast_to((128, C)))
            ones_m = consts.tile([128, 128], AMID, tag="ones")
            nc.vector.memset(ones_m[:], 1.0)
            if attn_f32:
                id_m = id_sb
            else:
                id_m = consts.tile([128, 128], BF16, tag="idm")
                nc.vector.tensor_copy(id_m[:], id_sb[:].bitcast(F32))
            if attn_f32:
                wq_m, wk_m, wv_m = wq_sb, wk_sb, wv_sb
            else:
                wq_m = consts.tile([128, NCHUNK, C], BF16, tag="wqm")
                wk_m = consts.tile([128, NCHUNK, C], BF16, tag="wkm")
                wv_m = consts.tile([128, NCHUNK, C], BF16, tag="wvm")
                for dst, src in ((wq_m, wq_sb), (wk_m, wk_sb), (wv_m, wv_sb)):
                    nc.vector.tensor_copy(dst[:], src[:].bitcast(F32))

            state = {}

            def front(b, X_pre=None):
                if X_pre is not None:
                    X_blk = X_pre
                else:
                    X_blk = px.tile([128, BLK, C], F32R, tag="x")
                    nc.sync.dma_start(X_blk[:], x_d[:, b * BLK:(b + 1) * BLK, :])
                if attn_f32 or xconv == 'none':
                    X_m, id_t, TDT = X_blk, id_sb, F32R
                else:
                    X_m = px.tile([128, BLK, C], BF16, tag="xm")
                    conv = (nc.gpsimd.tensor_copy if xconv == 'pool'
                            else nc.vector.tensor_copy)
                    conv(X_m[:], X_blk[:].bitcast(F32))
                    id_t, TDT = id_m, BF16
                XT_sb = pxt.tile([128, NCHUNK, BLK, 128], AMID, tag="xt")
                for s in range(BLK):
                    XT_ps = psf.tile([128, NCHUNK, 128], TDT, tag="f")
                    for jc in range(NCHUNK):
                        nc.tensor.transpose(
                            XT_ps[:, jc, :],
                            X_m[:, s, jc * 128:(jc + 1) * 128], id_t[:])
                    cp_engine(xt_copy)(XT_sb[:, :, s, :], XT_ps[:])
                QT = pqt.tile([128, NCHUNK, BLK, 128], AMID, tag="qt")
                KT = pqt.tile([128, NCHUNK, BLK, 128], AMID, tag="kt")
                for w_m, dst in ((wq_m, QT), (wk_m, KT)):
                    for co in range(NCHUNK):
                        PT = psf.tile([128, BLK * 128], F32, tag="f")
                        for jc in range(NCHUNK):
                            nc.tensor.matmul(
                                PT[:],
                                lhsT=w_m[:, jc, co * 128:(co + 1) * 128],
                                rhs=XT_sb[:, jc, :, :],
                                start=(jc == 0), stop=(jc == NCHUNK - 1))
                        cp_engine(qk_copy)(dst[:, co, :, :], PT[:])
                V = pv.tile([128, BLK, C], AMID, tag="v")
                for s in range(BLK):
                    VP = psf.tile([128, C], F32, tag="f")
                    for jc in range(NCHUNK):
                        nc.tensor.matmul(
                            VP[:], lhsT=XT_sb[:, jc, s, :],
                            rhs=wv_m[:, jc, :],
                            start=(jc == 0), stop=(jc == NCHUNK - 1))
                    cp_engine(v_copy)(V[:, s, :], VP[:])
                state[b] = (QT, KT, V)

            def back_seq(QT, KT, V, O_sb, s, p_s, p_b, t_s="s", t_b="b"):
                # S^T-direct: exp(S^T) IS A^T; softmax sums via ones-matmul
                # column-sum broadcast; normalization folded into OT psum exit.
                # Even heads (PE row-group 0) and odd heads (row-group 1)
                # run concurrently in the array -> MUST land in different
                # PSUM banks (same-bank concurrent row-group writes hang).
                S_e = p_s.tile([128, G // 2, 128], F32, tag=t_s)
                S_o = p_s.tile([128, G // 2, 128], F32, tag=t_s)
                # evens first so exp(S_e) can start while odd S still runs
                for g in (0, 2, 4, 6, 1, 3, 5, 7):
                    p0 = 64 * (g % 2)
                    S_ps = S_e if g % 2 == 0 else S_o
                    # lhsT=K^T, rhs=Q^T  ->  S^T[j, i]
                    nc.tensor.matmul(
                        S_ps[:, g // 2, :],
                        lhsT=KT[p0:p0 + 64, g // 2, s, :],
                        rhs=QT[p0:p0 + 64, g // 2, s, :],
                        start=True, stop=True)
                AT = pa.tile([128, G, 128], AMID, tag="at")
                nc.scalar.activation(AT[:, 0:G:2, :], S_e[:], EXP,
                                     scale=1.0 / np.sqrt(GP))
                nc.scalar.activation(AT[:, 1:G:2, :], S_o[:], EXP,
                                     scale=1.0 / np.sqrt(GP))
                # rcp_t[p, c, i] = 1/colsum[g = 2c + p//64, i]
                rcp_t = pstat.tile([128, NCHUNK, 128], F32, tag="rcpt")
                if sums == 'par':
                    # Pool partition all-reduce: every partition gets all
                    # heads' column sums; no PE work.
                    s_all = pstat.tile([128, G, 128], F32, tag="sall")
                    nc.gpsimd.partition_all_reduce(
                        s_all[:], AT[:], channels=128,
                        reduce_op=bass_isa.ReduceOp.add)
                    nc.vector.reciprocal(rcp_t[0:64, :, :],
                                         s_all[0:64, 0:G:2, :])
                    nc.vector.reciprocal(rcp_t[64:128, :, :],
                                         s_all[64:128, 1:G:2, :])
                else:
                    # Column-sum broadcast: bc1[p, c*128+i] = sum_j AT[j, 2c, i]
                    # (even heads), bc2 odd; every partition gets a copy.
                    bc1 = p_b.tile([128, NCHUNK, 128], F32, tag=t_b)
                    bc2 = p_b.tile([128, NCHUNK, 128], F32, tag=t_b)
                    nc.tensor.matmul(bc1[:], lhsT=ones_m[:],
                                     rhs=AT[:, 0:G:2, :], start=True, stop=True)
                    nc.tensor.matmul(bc2[:], lhsT=ones_m[:],
                                     rhs=AT[:, 1:G:2, :], start=True, stop=True)
                    nc.vector.reciprocal(rcp_t[0:64, :, :], bc1[0:64, :, :])
                    nc.vector.reciprocal(rcp_t[64:128, :, :], bc2[64:128, :, :])
                OT_ps = p_b.tile([128, NCHUNK, 128], F32, tag=t_b)
                for g in range(G):
                    p0 = 64 * (g % 2)
                    nc.tensor.matmul(
                        OT_ps[p0:p0 + 64, g // 2, :],
                        lhsT=V[:, s, 64 * g:64 * (g + 1)],
                        rhs=AT[:, g, :],
                        start=True, stop=True)
                OT_sb = pot.tile([128, NCHUNK, 128], F32R, tag="ot")
                if ot_mul == 'dve':
                    nc.vector.tensor_mul(OT_sb[:], OT_ps[:], rcp_t[:])
                else:
                    nc.gpsimd.tensor_mul(OT_sb[:], OT_ps[:], rcp_t[:])
                O_ps = p_b.tile([128, C], F32, tag=t_b)
                for jc in range(NCHUNK):
                    nc.tensor.matmul(
                        O_ps[:], lhsT=OT_sb[:, jc, :],
                        rhs=wo_sb[:, jc, :],
                        start=(jc == 0), stop=(jc == NCHUNK - 1))
                nc.vector.tensor_add(O_sb[:, s, :], O_ps[:], bo_sb[:])

            def back(b):
                QT, KT, V = state.pop(b)
                O_sb = po.tile([128, BLK, C], F32, tag="o")
                for s in range(BLK):
                    back_seq(QT, KT, V, O_sb, s, pss, psb)
                nc.sync.dma_start(out_d[:, b * BLK:(b + 1) * BLK, :], O_sb[:])

            def back_pair(b1, b2):
                # Tail: interleave the last two blocks' backs at seq
                # granularity so their dependency chains overlap. The second
                # block's psum tiles borrow the front pool (idle in the tail).
                QT1, KT1, V1 = state.pop(b1)
                QT2, KT2, V2 = state.pop(b2)
                O1 = po.tile([128, BLK, C], F32, tag="o")
                O2 = po.tile([128, BLK, C], F32, tag="o")
                for s in range(BLK):
                    back_seq(QT1, KT1, V1, O1, s, pss, psb)
                    back_seq(QT2, KT2, V2, O2, s, psf, psf, t_s="f", t_b="f")
                nc.sync.dma_start(out_d[:, b1 * BLK:(b1 + 1) * BLK, :], O1[:])
                nc.sync.dma_start(out_d[:, b2 * BLK:(b2 + 1) * BLK, :], O2[:])

            def whole(X_first=None):
                for b in range(nblk + 1):
                    if b < nblk:
                        front(b, X_pre=(X_first if b == 0 else None))
                    if b >= 1:
                        if nblk >= 2 and b - 1 == nblk - 2:
                            continue  # deferred into the tail pair
                        elif b - 1 == nblk - 1 and nblk >= 2:
                            back_pair(nblk - 2, nblk - 1)
                        else:
                            back(b - 1)

            if reps == 1:
                whole(X_first=X0)
            else:
                with tc.For_i(0, reps, 1):
                    whole()

    nc.compile()
    return nc


def make_in_maps(x, Wq, Wk, Wv, Wo, bo, num_cores=8):
    """Full inputs -> per-core input dicts (data-parallel over N)."""
    x = np.asarray(x, dtype=np.float32)
    ident = np.eye(128, dtype=np.float32)
    wqT = np.ascontiguousarray(np.asarray(Wq, np.float32).T)
    wkT = np.ascontiguousarray(np.asarray(Wk, np.float32).T)
    wvT = np.ascontiguousarray(np.asarray(Wv, np.float32).T)
    woT = np.ascontiguousarray(np.asarray(Wo, np.float32).T)
    bo = np.asarray(bo, np.float32)
    return [{"x": np.ascontiguousarray(x[n]), "wqT": wqT, "wkT": wkT,
             "wvT": wvT, "woT": woT, "bo": bo, "ident": ident}
            for n in range(num_cores)]


_NC_CACHE = {}

# Single source of truth for the shipped configuration (test.py reads this).
KERNEL_CFG = dict(qk_copy='act', v_copy='act', xt_copy='dve',
                  psum_bufs=(4, 2, 2))


def kernel(x, Wq, Wk, Wv, Wo, bo):
    import numpy as np
    from concourse import bass_utils

    if "nc" not in _NC_CACHE:
        _NC_CACHE["nc"] = build_kernel(num_cores=8, **KERNEL_CFG)
    nc = _NC_CACHE["nc"]
    in_maps = make_in_maps(x, Wq, Wk, Wv, Wo, bo, num_cores=8)
    res = bass_utils.run_bass_kernel_spmd(nc, in_maps, core_ids=list(range(8)))
    out = np.stack([res.results[c]["out"] for c in range(8)], axis=0)
    return out.astype(np.float32)


# revision 29
# speedup vs baseline: 8.0918x; 1.6209x over previous
"""Self-contained TRN2 Bass kernel for axial attention (nn_AxialAttention).

kernel(**inputs) takes FULL inputs (x [8,128,128,512], Wq/Wk/Wv/Wo [512,512],
bo [512]) and returns the FULL output [8,128,128,512] (float32).

Sharding: data-parallel over N across 8 NeuronCores (core c computes image c).

Per-core dataflow (v2), per block of BLK=4 sequences (seq = one W column,
tokens = H axis):
  X_blk  [128 t, 4 s, 512 c] f32  <- DMA
  X^T    [128 c_loc, 4 jc, 4 s, 128 t] bf16, PE transposes (f32r in, bf16 out)
  Q^T/K^T [128 co_loc, 4 co, 4 s, 128 t] bf16 = Wq/k^T.T @ X^T, psum exit on ACT
  V      [128 t, 4 s, 512 c] bf16 = X^T.T @ Wv^T, psum exit on ACT
  S^T_g  [128 j, 128 i] psum = K^T_g.T @ Q^T_g   (even/odd heads -> 2 banks)
  A^T    exp(S^T/8) directly (no transposes; softmax max-subtraction skipped:
         logits ~N(0,1), |S|<~6, exp safe in f32)
  colsum broadcast: ones^T @ A^T (per parity) -> bc psum [128, 4, 128]
  rcp_t  1/bc on the used partition halves (DVE, 2 ops)
  OT     [128 c_loc, 4 jc, 128 i] psum = V_g.T...: lhsT=V_g, rhs=A^T_g
  OT_sb  = OT * rcp_t  (softmax normalization folded into psum exit, DVE)
  O      [128 t, 512 co] = OT.T @ Wo^T (f32r) + bo (DVE psum exit)
"""
import sys
sys.path.insert(0, "/opt/trn_rl_repo")
sys.path.insert(0, "/root/.axon_site/_ro/trn_rl_repo")

import numpy as np

import concourse.bass as bass
import concourse.bacc as bacc
import concourse.tile as tile
from concourse import mybir
from concourse import bass_isa

F32 = mybir.dt.float32
F32R = mybir.dt.float32r
BF16 = mybir.dt.bfloat16
EXP = mybir.ActivationFunctionType.Exp

H = 128   # tokens per sequence (attention axis)
W = 128   # sequences per core
C = 512
G = 8     # heads
GP = C // G  # 64
BLK = 4   # sequences per block
NCHUNK = C // 128  # 4 k-chunks


def build_kernel(num_cores=8, attn_f32=False, w_total=W, reps=1,
                 qk_copy='dve', v_copy='dve', xt_copy='dve',
                 ot_mul='dve', xconv='pool', psum_bufs=(3, 2, 3), sbufs=None,
                 psum_unified=False, pre_x0=True, sums='bc', attn_mode='std',
                 an_engine='dve', at_copy='act', ot_copy='dve',
                 exp_mode='batched', sums_engine='pool',
                 pt_bf16=False, o_direct=False, ot_bf16=False):
    """Build + compile the Bass module. Returns nc.

    reps>1 wraps the computation in a dynamic loop (for wall-clock
    rep-differencing timing; results identical, just recomputed)."""
    nblk = w_total // BLK
    nc = bacc.Bacc("TRN2", target_bir_lowering=False, debug=False,
                   num_devices=num_cores)

    x_d = nc.dram_tensor("x", [H, w_total, C], F32R, kind="ExternalInput").ap()
    wq_d = nc.dram_tensor("wqT", [C, C], F32R, kind="ExternalInput").ap()
    wk_d = nc.dram_tensor("wkT", [C, C], F32R, kind="ExternalInput").ap()
    wv_d = nc.dram_tensor("wvT", [C, C], F32R, kind="ExternalInput").ap()
    wo_d = nc.dram_tensor("woT", [C, C], F32R, kind="ExternalInput").ap()
    bo_d = nc.dram_tensor("bo", [C], F32, kind="ExternalInput").ap()
    id_d = nc.dram_tensor("ident", [128, 128], F32R, kind="ExternalInput").ap()
    out_d = nc.dram_tensor("out", [H, w_total, C], F32, kind="ExternalOutput").ap()

    AMID = F32R if attn_f32 else BF16

    sb_bufs = sbufs or {}
    def B(name, d):
        return sb_bufs.get(name, d)

    def cp_engine(which):
        return nc.scalar.copy if which == 'act' else nc.vector.tensor_copy

    with tile.TileContext(nc) as tc:
        with tc.tile_pool(name="consts", bufs=1) as consts, \
             tc.tile_pool(name="px", bufs=B('px', 3)) as px, \
             tc.tile_pool(name="pxt", bufs=B('pxt', 3)) as pxt, \
             tc.tile_pool(name="pqt", bufs=B('pqt', 3)) as pqt, \
             tc.tile_pool(name="pv", bufs=B('pv', 3)) as pv, \
             tc.tile_pool(name="pa", bufs=B('pa', 4)) as pa, \
             tc.tile_pool(name="pstat", bufs=B('pstat', 4)) as pstat, \
             tc.tile_pool(name="pot", bufs=B('pot', 3)) as pot, \
             tc.tile_pool(name="po", bufs=B('po', 3)) as po, \
             tc.tile_pool(name="psf", bufs=(8 if psum_unified else psum_bufs[0]),
                          space="PSUM") as psf, \
             tc.tile_pool(name="pss", bufs=psum_bufs[1], space="PSUM") as _pss, \
             tc.tile_pool(name="psb", bufs=psum_bufs[2], space="PSUM") as _psb:
            if psum_unified:
                class _U:
                    _n = [0]
                    def tile(self, shape, dtype, tag=None):
                        self._n[0] += 1
                        return psf.tile(shape, dtype, tag="f",
                                        name=f"u{self._n[0]}")
                pss = psb = _U()
            else:
                pss, psb = _pss, _psb

            # ---- block-0 input DMA first: the first PE work (transposes of
            # X0) must not wait behind 4MB of weight DMAs in the queue ----
            X0 = None
            if pre_x0 and reps == 1:
                X0 = px.tile([128, BLK, C], F32R, tag="x")
                nc.sync.dma_start(X0[:], x_d[:, 0:BLK, :])

            # ---- constants (id first: the first PE work -- transposes of X0
            # -- needs only X0 + id, not the 4MB of weights) ----
            id_sb = consts.tile([128, 128], F32R, tag="id")
            nc.sync.dma_start(id_sb[:], id_d[:])
            wq_sb = consts.tile([128, NCHUNK, C], F32R, tag="wq")
            wk_sb = consts.tile([128, NCHUNK, C], F32R, tag="wk")
            wv_sb = consts.tile([128, NCHUNK, C], F32R, tag="wv")
            wo_sb = consts.tile([128, NCHUNK, C], F32R, tag="wo")
            for w_sb, w_d in ((wq_sb, wq_d), (wk_sb, wk_d), (wv_sb, wv_d),
                              (wo_sb, wo_d)):
                nc.sync.dma_start(w_sb[:], w_d.rearrange("(j p) c -> p j c", p=128))
            bo_sb = consts.tile([128, C], F32, tag="bo")
            nc.sync.dma_start(
                bo_sb[:],
                bo_d.rearrange("(o c) -> o c", o=1).broadcast_to((128, C)))
            ones_m = consts.tile([128, 128], AMID, tag="ones")
            nc.vector.memset(ones_m[:], 1.0)
            if o_direct:
                ones1 = consts.tile([1, 128], F32R, tag="ones1")
                nc.vector.memset(ones1[:], 1.0)
                bo_r = consts.tile([1, C], F32R, tag="bor")
                nc.sync.dma_start(bo_r[:].bitcast(F32),
                                  bo_d.rearrange("(o c) -> o c", o=1))
            if attn_f32:
                id_m = id_sb
            else:
                id_m = consts.tile([128, 128], BF16, tag="idm")
                nc.vector.tensor_copy(id_m[:], id_sb[:].bitcast(F32))
            if attn_f32:
                wq_m, wk_m, wv_m = wq_sb, wk_sb, wv_sb
            else:
                wq_m = consts.tile([128, NCHUNK, C], BF16, tag="wqm")
                wk_m = consts.tile([128, NCHUNK, C], BF16, tag="wkm")
                wv_m = consts.tile([128, NCHUNK, C], BF16, tag="wvm")
                for dst, src in ((wq_m, wq_sb), (wk_m, wk_sb), (wv_m, wv_sb)):
                    nc.vector.tensor_copy(dst[:], src[:].bitcast(F32))
            wo_m = wo_sb
            if ot_bf16 and not attn_f32:
                wo_m = consts.tile([128, NCHUNK, C], BF16, tag="wom")
                nc.vector.tensor_copy(wo_m[:], wo_sb[:].bitcast(F32))

            state = {}

            def front(b, X_pre=None):
                if X_pre is not None:
                    X_blk = X_pre
                else:
                    X_blk = px.tile([128, BLK, C], F32R, tag="x")
                    nc.sync.dma_start(X_blk[:], x_d[:, b * BLK:(b + 1) * BLK, :])
                if attn_f32 or xconv == 'none':
                    X_m, id_t, TDT = X_blk, id_sb, F32R
                else:
                    X_m = px.tile([128, BLK, C], BF16, tag="xm")
                    conv = (nc.gpsimd.tensor_copy if xconv == 'pool'
                            else nc.vector.tensor_copy)
                    conv(X_m[:], X_blk[:].bitcast(F32))
                    id_t, TDT = id_m, BF16
                XT_sb = pxt.tile([128, NCHUNK, BLK, 128], AMID, tag="xt")
                for s in range(BLK):
                    XT_ps = psf.tile([128, NCHUNK, 128], TDT, tag="f")
                    for jc in range(NCHUNK):
                        nc.tensor.transpose(
                            XT_ps[:, jc, :],
                            X_m[:, s, jc * 128:(jc + 1) * 128], id_t[:])
                    cp_engine(xt_copy)(XT_sb[:, :, s, :], XT_ps[:])
                QT = pqt.tile([128, NCHUNK, BLK, 128], AMID, tag="qt")
                KT = pqt.tile([128, NCHUNK, BLK, 128], AMID, tag="kt")
                PDT = BF16 if (pt_bf16 and not attn_f32) else F32
                for w_m, dst in ((wq_m, QT), (wk_m, KT)):
                    for co in range(NCHUNK):
                        PT = psf.tile([128, BLK * 128], PDT, tag="f")
                        for jc in range(NCHUNK):
                            nc.tensor.matmul(
                                PT[:],
                                lhsT=w_m[:, jc, co * 128:(co + 1) * 128],
                                rhs=XT_sb[:, jc, :, :],
                                start=(jc == 0), stop=(jc == NCHUNK - 1))
                        cp_engine(qk_copy)(dst[:, co, :, :], PT[:])
                V = pv.tile([128, BLK, C], AMID, tag="v")
                for s in range(BLK):
                    VP = psf.tile([128, C], PDT, tag="f")
                    for jc in range(NCHUNK):
                        nc.tensor.matmul(
                            VP[:], lhsT=XT_sb[:, jc, s, :],
                            rhs=wv_m[:, jc, :],
                            start=(jc == 0), stop=(jc == NCHUNK - 1))
                    cp_engine(v_copy)(V[:, s, :], VP[:])
                state[b] = (QT, KT, V)

            def back_seq_std(QT, KT, V, O_sb, s, p_s, p_b, t_s="s", t_b="b",
                             blk_idx=None):
                # std orientation: S[i, j]; exp with ACT-accumulated row sums;
                # A normalized (DVE/Pool mul), A^T via PE transposes.
                S_e = p_s.tile([128, G // 2, 128], F32, tag=t_s)
                S_o = p_s.tile([128, G // 2, 128], F32, tag=t_s)
                for g in range(G):
                    p0 = 64 * (g % 2)
                    S_ps = S_e if g % 2 == 0 else S_o
                    # lhsT=Q^T, rhs=K^T  ->  S[i, j]
                    nc.tensor.matmul(
                        S_ps[:, g // 2, :],
                        lhsT=QT[p0:p0 + 64, g // 2, s, :],
                        rhs=KT[p0:p0 + 64, g // 2, s, :],
                        start=True, stop=True)
                A = pa.tile([128, G, 128], AMID, tag="a")
                ssum = pstat.tile([128, G], F32, tag="sums")
                if exp_mode == 'perhead':
                    # 8 small ACT ops; row sums accumulated for free
                    for g in range(G):
                        S_ps = S_e if g % 2 == 0 else S_o
                        nc.scalar.activation(
                            A[:, g, :], S_ps[:, g // 2, :], EXP,
                            scale=1.0 / np.sqrt(GP),
                            accum_out=ssum[:, g:g + 1])
                else:
                    # 2 big ACT ops; row sums via free-axis reduce elsewhere
                    a_ev = A[:, 0:G:2, :]
                    a_od = A[:, 1:G:2, :]
                    nc.scalar.activation(a_ev, S_e[:], EXP,
                                         scale=1.0 / np.sqrt(GP))
                    nc.scalar.activation(a_od, S_o[:], EXP,
                                         scale=1.0 / np.sqrt(GP))
                    se = ssum[:, 0:G:2].rearrange("p (g o) -> p g o", o=1)
                    so = ssum[:, 1:G:2].rearrange("p (g o) -> p g o", o=1)
                    red = (nc.gpsimd if sums_engine == 'pool'
                           else nc.vector)
                    red.reduce_sum(se, a_ev, axis=mybir.AxisListType.X)
                    red.reduce_sum(so, a_od, axis=mybir.AxisListType.X)
                rcp = pstat.tile([128, G], F32, tag="rcp")
                nc.vector.reciprocal(rcp[:], ssum[:])
                AN = pa.tile([128, G, 128], AMID, tag="an")
                rcp_b = rcp[:].rearrange("p (o g) -> p o g", o=1) \
                    .rearrange("p o g -> p g o") \
                    .broadcast_to((128, G, 128))
                if an_engine == 'pool':
                    nc.gpsimd.tensor_mul(AN[:], A[:], rcp_b)
                else:
                    nc.vector.tensor_mul(AN[:], A[:], rcp_b)
                AT = pa.tile([128, G, 128], AMID, tag="at")
                idt = id_m if not attn_f32 else id_sb
                for half in range(2):
                    ATp = p_b.tile([128, G // 2, 128], AMID, tag=t_b)
                    for gg in range(G // 2):
                        g = half * (G // 2) + gg
                        nc.tensor.transpose(ATp[:, gg, :], AN[:, g, :], idt[:])
                    cp_engine(at_copy)(
                        AT[:, half * (G // 2):(half + 1) * (G // 2), :],
                        ATp[:])
                ODT = BF16 if (ot_bf16 and not attn_f32) else F32
                OT_ps = p_b.tile([128, NCHUNK, 128], ODT, tag=t_b)
                for g in range(G):
                    p0 = 64 * (g % 2)
                    nc.tensor.matmul(
                        OT_ps[p0:p0 + 64, g // 2, :],
                        lhsT=V[:, s, 64 * g:64 * (g + 1)],
                        rhs=AT[:, g, :],
                        start=True, stop=True)
                OT_sb = pot.tile([128, NCHUNK, 128],
                                 BF16 if ODT == BF16 else F32R, tag="ot")
                cp_engine(ot_copy)(OT_sb[:], OT_ps[:])
                wo_u = wo_m if ODT == BF16 else wo_sb
                O_ps = p_b.tile([128, C], F32, tag=t_b)
                for jc in range(NCHUNK):
                    nc.tensor.matmul(
                        O_ps[:], lhsT=OT_sb[:, jc, :],
                        rhs=wo_u[:, jc, :],
                        start=(jc == 0),
                        stop=(jc == NCHUNK - 1 and not o_direct))
                if o_direct:
                    # += 1*bo via a k=1 accumulation step, then DMA the
                    # finished rows straight from PSUM to DRAM (no O_sb).
                    nc.tensor.matmul(O_ps[:], lhsT=ones1[:], rhs=bo_r[:],
                                     start=False, stop=True)
                    nc.sync.dma_start(
                        out_d[:, blk_idx * BLK + s, :], O_ps[:])
                else:
                    nc.vector.tensor_add(O_sb[:, s, :], O_ps[:], bo_sb[:])

            def back_seq_st2(QT, KT, V, O_sb, s, p_s, p_b, t_s="s", t_b="b",
                             blk_idx=None):
                # S^T-direct: exp(S^T) IS A^T; softmax sums via ones-matmul
                # column-sum broadcast; normalization folded into OT psum exit.
                # Even heads (PE row-group 0) and odd heads (row-group 1)
                # run concurrently in the array -> MUST land in different
                # PSUM banks (same-bank concurrent row-group writes hang).
                S_e = p_s.tile([128, G // 2, 128], F32, tag=t_s)
                S_o = p_s.tile([128, G // 2, 128], F32, tag=t_s)
                # evens first so exp(S_e) can start while odd S still runs
                for g in (0, 2, 4, 6, 1, 3, 5, 7):
                    p0 = 64 * (g % 2)
                    S_ps = S_e if g % 2 == 0 else S_o
                    # lhsT=K^T, rhs=Q^T  ->  S^T[j, i]
                    nc.tensor.matmul(
                        S_ps[:, g // 2, :],
                        lhsT=KT[p0:p0 + 64, g // 2, s, :],
                        rhs=QT[p0:p0 + 64, g // 2, s, :],
                        start=True, stop=True)
                AT = pa.tile([128, G, 128], AMID, tag="at")
                nc.scalar.activation(AT[:, 0:G:2, :], S_e[:], EXP,
                                     scale=1.0 / np.sqrt(GP))
                nc.scalar.activation(AT[:, 1:G:2, :], S_o[:], EXP,
                                     scale=1.0 / np.sqrt(GP))
                # rcp_t[p, c, i] = 1/colsum[g = 2c + p//64, i]
                rcp_t = pstat.tile([128, NCHUNK, 128], F32, tag="rcpt")
                if sums == 'par':
                    # Pool partition all-reduce: every partition gets all
                    # heads' column sums; no PE work.
                    s_all = pstat.tile([128, G, 128], F32, tag="sall")
                    nc.gpsimd.partition_all_reduce(
                        s_all[:], AT[:], channels=128,
                        reduce_op=bass_isa.ReduceOp.add)
                    nc.vector.reciprocal(rcp_t[0:64, :, :],
                                         s_all[0:64, 0:G:2, :])
                    nc.vector.reciprocal(rcp_t[64:128, :, :],
                                         s_all[64:128, 1:G:2, :])
                else:
                    # Column-sum broadcast: bc1[p, c*128+i] = sum_j AT[j, 2c, i]
                    # (even heads), bc2 odd; every partition gets a copy.
                    bc1 = p_b.tile([128, NCHUNK, 128], F32, tag=t_b)
                    bc2 = p_b.tile([128, NCHUNK, 128], F32, tag=t_b)
                    nc.tensor.matmul(bc1[:], lhsT=ones_m[:],
                                     rhs=AT[:, 0:G:2, :], start=True, stop=True)
                    nc.tensor.matmul(bc2[:], lhsT=ones_m[:],
                                     rhs=AT[:, 1:G:2, :], start=True, stop=True)
                    nc.vector.reciprocal(rcp_t[0:64, :, :], bc1[0:64, :, :])
                    nc.vector.reciprocal(rcp_t[64:128, :, :], bc2[64:128, :, :])
                OT_ps = p_b.tile([128, NCHUNK, 128], F32, tag=t_b)
                for g in range(G):
                    p0 = 64 * (g % 2)
                    nc.tensor.matmul(
                        OT_ps[p0:p0 + 64, g // 2, :],
                        lhsT=V[:, s, 64 * g:64 * (g + 1)],
                        rhs=AT[:, g, :],
                        start=True, stop=True)
                OT_sb = pot.tile([128, NCHUNK, 128], F32R, tag="ot")
                if ot_mul == 'dve':
                    nc.vector.tensor_mul(OT_sb[:], OT_ps[:], rcp_t[:])
                else:
                    nc.gpsimd.tensor_mul(OT_sb[:], OT_ps[:], rcp_t[:])
                O_ps = p_b.tile([128, C], F32, tag=t_b)
                for jc in range(NCHUNK):
                    nc.tensor.matmul(
                        O_ps[:], lhsT=OT_sb[:, jc, :],
                        rhs=wo_sb[:, jc, :],
                        start=(jc == 0), stop=(jc == NCHUNK - 1))
                nc.vector.tensor_add(O_sb[:, s, :], O_ps[:], bo_sb[:])

            back_seq = back_seq_std if attn_mode == 'std' else back_seq_st2

            def back(b):
                QT, KT, V = state.pop(b)
                O_sb = None if o_direct else po.tile([128, BLK, C], F32,
                                                     tag="o")
                for s in range(BLK):
                    back_seq(QT, KT, V, O_sb, s, pss, psb, blk_idx=b)
                if not o_direct:
                    nc.sync.dma_start(out_d[:, b * BLK:(b + 1) * BLK, :],
                                      O_sb[:])

            def back_pair(b1, b2):
                # Tail: interleave the last two blocks' backs at seq
                # granularity so their dependency chains overlap. The second
                # block's psum tiles borrow the front pool (idle in the tail).
                QT1, KT1, V1 = state.pop(b1)
                QT2, KT2, V2 = state.pop(b2)
                O1 = None if o_direct else po.tile([128, BLK, C], F32,
                                                    tag="o")
                O2 = None if o_direct else po.tile([128, BLK, C], F32,
                                                   tag="o")
                for s in range(BLK):
                    back_seq(QT1, KT1, V1, O1, s, pss, psb, blk_idx=b1)
                    back_seq(QT2, KT2, V2, O2, s, psf, psf, t_s="f", t_b="f",
                             blk_idx=b2)
                if not o_direct:
                    nc.sync.dma_start(out_d[:, b1 * BLK:(b1 + 1) * BLK, :],
                                      O1[:])
                    nc.sync.dma_start(out_d[:, b2 * BLK:(b2 + 1) * BLK, :],
                                      O2[:])

            def whole(X_first=None, pair_tail=True):
                # pair_tail only helps the drain of a single-shot run; in a
                # rep loop the next iteration's fronts fill the tail anyway,
                # and the borrowed front-pool psum banks would contend.
                for b in range(nblk + 1):
                    if b < nblk:
                        front(b, X_pre=(X_first if b == 0 else None))
                    if b >= 1:
                        if pair_tail and nblk >= 2 and b - 1 == nblk - 2:
                            continue  # deferred into the tail pair
                        elif pair_tail and b - 1 == nblk - 1 and nblk >= 2:
                            back_pair(nblk - 2, nblk - 1)
                        else:
                            back(b - 1)

            if reps == 1:
                whole(X_first=X0)
            else:
                with tc.For_i(0, reps, 1):
                    whole(pair_tail=False)

    nc.compile()
    return nc


def make_in_maps(x, Wq, Wk, Wv, Wo, bo, num_cores=8):
    """Full inputs -> per-core input dicts (data-parallel over N)."""
    x = np.asarray(x, dtype=np.float32)
    ident = np.eye(128, dtype=np.float32)
    wqT = np.ascontiguousarray(np.asarray(Wq, np.float32).T)
    wkT = np.ascontiguousarray(np.asarray(Wk, np.float32).T)
    wvT = np.ascontiguousarray(np.asarray(Wv, np.float32).T)
    woT = np.ascontiguousarray(np.asarray(Wo, np.float32).T)
    bo = np.asarray(bo, np.float32)
    return [{"x": np.ascontiguousarray(x[n]), "wqT": wqT, "wkT": wkT,
             "wvT": wvT, "woT": woT, "bo": bo, "ident": ident}
            for n in range(num_cores)]


_NC_CACHE = {}

# Single source of truth for the shipped configuration (test.py reads this).
KERNEL_CFG = dict(attn_mode='std', xconv='none', qk_copy='dve',
                  v_copy='dve', xt_copy='dve', an_engine='pool',
                  at_copy='dve', ot_copy='dve', exp_mode='perhead',
                  psum_bufs=(2, 3, 3),
                  sbufs={'px': 3, 'pxt': 3, 'pqt': 3, 'pv': 3, 'pa': 4,
                         'pstat': 6, 'pot': 3, 'po': 3})


def kernel(x, Wq, Wk, Wv, Wo, bo):
    import numpy as np
    from concourse import bass_utils

    if "nc" not in _NC_CACHE:
        _NC_CACHE["nc"] = build_kernel(num_cores=8, **KERNEL_CFG)
    nc = _NC_CACHE["nc"]
    in_maps = make_in_maps(x, Wq, Wk, Wv, Wo, bo, num_cores=8)
    res = bass_utils.run_bass_kernel_spmd(nc, in_maps, core_ids=list(range(8)))
    out = np.stack([res.results[c]["out"] for c in range(8)], axis=0)
    return out.astype(np.float32)


# revision 31
# speedup vs baseline: 8.1555x; 1.0079x over previous
"""Self-contained TRN2 Bass kernel for axial attention (nn_AxialAttention).

kernel(**inputs) takes FULL inputs (x [8,128,128,512], Wq/Wk/Wv/Wo [512,512],
bo [512]) and returns the FULL output [8,128,128,512] (float32).

Sharding: data-parallel over N across 8 NeuronCores (core c computes image c).

Per-core dataflow (v2), per block of BLK=4 sequences (seq = one W column,
tokens = H axis):
  X_blk  [128 t, 4 s, 512 c] f32  <- DMA
  X^T    [128 c_loc, 4 jc, 4 s, 128 t] bf16, PE transposes (f32r in, bf16 out)
  Q^T/K^T [128 co_loc, 4 co, 4 s, 128 t] bf16 = Wq/k^T.T @ X^T, psum exit on ACT
  V      [128 t, 4 s, 512 c] bf16 = X^T.T @ Wv^T, psum exit on ACT
  S^T_g  [128 j, 128 i] psum = K^T_g.T @ Q^T_g   (even/odd heads -> 2 banks)
  A^T    exp(S^T/8) directly (no transposes; softmax max-subtraction skipped:
         logits ~N(0,1), |S|<~6, exp safe in f32)
  colsum broadcast: ones^T @ A^T (per parity) -> bc psum [128, 4, 128]
  rcp_t  1/bc on the used partition halves (DVE, 2 ops)
  OT     [128 c_loc, 4 jc, 128 i] psum = V_g.T...: lhsT=V_g, rhs=A^T_g
  OT_sb  = OT * rcp_t  (softmax normalization folded into psum exit, DVE)
  O      [128 t, 512 co] = OT.T @ Wo^T (f32r) + bo (DVE psum exit)
"""
import sys
sys.path.insert(0, "/opt/trn_rl_repo")
sys.path.insert(0, "/root/.axon_site/_ro/trn_rl_repo")

import numpy as np

import concourse.bass as bass
import concourse.bacc as bacc
import concourse.tile as tile
from concourse import mybir
from concourse import bass_isa

F32 = mybir.dt.float32
F32R = mybir.dt.float32r
BF16 = mybir.dt.bfloat16
EXP = mybir.ActivationFunctionType.Exp

H = 128   # tokens per sequence (attention axis)
W = 128   # sequences per core
C = 512
G = 8     # heads
GP = C // G  # 64
BLK = 4   # sequences per block
NCHUNK = C // 128  # 4 k-chunks


def build_kernel(num_cores=8, attn_f32=False, w_total=W, reps=1,
                 qk_copy='dve', v_copy='dve', xt_copy='dve',
                 ot_mul='dve', xconv='pool', psum_bufs=(3, 2, 3), sbufs=None,
                 psum_unified=False, pre_x0=True, sums='bc', attn_mode='std',
                 an_engine='dve', at_copy='act', ot_copy='dve',
                 exp_mode='batched', sums_engine='pool',
                 pt_bf16=False, o_direct=False, ot_bf16=False,
                 at_merge=False):
    """Build + compile the Bass module. Returns nc.

    reps>1 wraps the computation in a dynamic loop (for wall-clock
    rep-differencing timing; results identical, just recomputed)."""
    nblk = w_total // BLK
    nc = bacc.Bacc("TRN2", target_bir_lowering=False, debug=False,
                   num_devices=num_cores)

    x_d = nc.dram_tensor("x", [H, w_total, C], F32R, kind="ExternalInput").ap()
    wq_d = nc.dram_tensor("wqT", [C, C], F32R, kind="ExternalInput").ap()
    wk_d = nc.dram_tensor("wkT", [C, C], F32R, kind="ExternalInput").ap()
    wv_d = nc.dram_tensor("wvT", [C, C], F32R, kind="ExternalInput").ap()
    wo_d = nc.dram_tensor("woT", [C, C], F32R, kind="ExternalInput").ap()
    bo_d = nc.dram_tensor("bo", [C], F32, kind="ExternalInput").ap()
    id_d = nc.dram_tensor("ident", [128, 128], F32R, kind="ExternalInput").ap()
    out_d = nc.dram_tensor("out", [H, w_total, C], F32, kind="ExternalOutput").ap()

    AMID = F32R if attn_f32 else BF16

    sb_bufs = sbufs or {}
    def B(name, d):
        return sb_bufs.get(name, d)

    def cp_engine(which):
        return nc.scalar.copy if which == 'act' else nc.vector.tensor_copy

    with tile.TileContext(nc) as tc:
        with tc.tile_pool(name="consts", bufs=1) as consts, \
             tc.tile_pool(name="px", bufs=B('px', 3)) as px, \
             tc.tile_pool(name="pxt", bufs=B('pxt', 3)) as pxt, \
             tc.tile_pool(name="pqt", bufs=B('pqt', 3)) as pqt, \
             tc.tile_pool(name="pv", bufs=B('pv', 3)) as pv, \
             tc.tile_pool(name="pa", bufs=B('pa', 4)) as pa, \
             tc.tile_pool(name="pstat", bufs=B('pstat', 4)) as pstat, \
             tc.tile_pool(name="pot", bufs=B('pot', 3)) as pot, \
             tc.tile_pool(name="po", bufs=B('po', 3)) as po, \
             tc.tile_pool(name="psf", bufs=(8 if psum_unified else psum_bufs[0]),
                          space="PSUM") as psf, \
             tc.tile_pool(name="pss", bufs=psum_bufs[1], space="PSUM") as _pss, \
             tc.tile_pool(name="psb", bufs=psum_bufs[2], space="PSUM") as _psb:
            if psum_unified:
                class _U:
                    _n = [0]
                    def tile(self, shape, dtype, tag=None):
                        self._n[0] += 1
                        return psf.tile(shape, dtype, tag="f",
                                        name=f"u{self._n[0]}")
                pss = psb = _U()
            else:
                pss, psb = _pss, _psb

            # ---- block-0 input DMA first: the first PE work (transposes of
            # X0) must not wait behind 4MB of weight DMAs in the queue ----
            X0 = None
            if pre_x0 and reps == 1:
                X0 = px.tile([128, BLK, C], F32R, tag="x")
                nc.sync.dma_start(X0[:], x_d[:, 0:BLK, :])

            # ---- constants (id first: the first PE work -- transposes of X0
            # -- needs only X0 + id, not the 4MB of weights) ----
            id_sb = consts.tile([128, 128], F32R, tag="id")
            nc.sync.dma_start(id_sb[:], id_d[:])
            wq_sb = consts.tile([128, NCHUNK, C], F32R, tag="wq")
            wk_sb = consts.tile([128, NCHUNK, C], F32R, tag="wk")
            wv_sb = consts.tile([128, NCHUNK, C], F32R, tag="wv")
            wo_sb = consts.tile([128, NCHUNK, C], F32R, tag="wo")
            for w_sb, w_d in ((wq_sb, wq_d), (wk_sb, wk_d), (wv_sb, wv_d),
                              (wo_sb, wo_d)):
                nc.sync.dma_start(w_sb[:], w_d.rearrange("(j p) c -> p j c", p=128))
            bo_sb = consts.tile([128, C], F32, tag="bo")
            nc.sync.dma_start(
                bo_sb[:],
                bo_d.rearrange("(o c) -> o c", o=1).broadcast_to((128, C)))
            ones_m = consts.tile([128, 128], AMID, tag="ones")
            nc.vector.memset(ones_m[:], 1.0)
            if o_direct:
                ones1 = consts.tile([1, 128], F32R, tag="ones1")
                nc.vector.memset(ones1[:], 1.0)
                bo_r = consts.tile([1, C], F32R, tag="bor")
                nc.sync.dma_start(bo_r[:].bitcast(F32),
                                  bo_d.rearrange("(o c) -> o c", o=1))
            if attn_f32:
                id_m = id_sb
            else:
                id_m = consts.tile([128, 128], BF16, tag="idm")
                nc.vector.tensor_copy(id_m[:], id_sb[:].bitcast(F32))
            if attn_f32:
                wq_m, wk_m, wv_m = wq_sb, wk_sb, wv_sb
            else:
                wq_m = consts.tile([128, NCHUNK, C], BF16, tag="wqm")
                wk_m = consts.tile([128, NCHUNK, C], BF16, tag="wkm")
                wv_m = consts.tile([128, NCHUNK, C], BF16, tag="wvm")
                for dst, src in ((wq_m, wq_sb), (wk_m, wk_sb), (wv_m, wv_sb)):
                    nc.vector.tensor_copy(dst[:], src[:].bitcast(F32))
            wo_m = wo_sb
            if ot_bf16 and not attn_f32:
                wo_m = consts.tile([128, NCHUNK, C], BF16, tag="wom")
                nc.vector.tensor_copy(wo_m[:], wo_sb[:].bitcast(F32))

            state = {}

            def front(b, X_pre=None):
                if X_pre is not None:
                    X_blk = X_pre
                else:
                    X_blk = px.tile([128, BLK, C], F32R, tag="x")
                    nc.sync.dma_start(X_blk[:], x_d[:, b * BLK:(b + 1) * BLK, :])
                if attn_f32 or xconv == 'none':
                    X_m, id_t, TDT = X_blk, id_sb, F32R
                else:
                    X_m = px.tile([128, BLK, C], BF16, tag="xm")
                    conv = (nc.gpsimd.tensor_copy if xconv == 'pool'
                            else nc.vector.tensor_copy)
                    conv(X_m[:], X_blk[:].bitcast(F32))
                    id_t, TDT = id_m, BF16
                XT_sb = pxt.tile([128, NCHUNK, BLK, 128], AMID, tag="xt")
                for s in range(BLK):
                    XT_ps = psf.tile([128, NCHUNK, 128], TDT, tag="f")
                    for jc in range(NCHUNK):
                        nc.tensor.transpose(
                            XT_ps[:, jc, :],
                            X_m[:, s, jc * 128:(jc + 1) * 128], id_t[:])
                    cp_engine(xt_copy)(XT_sb[:, :, s, :], XT_ps[:])
                QT = pqt.tile([128, NCHUNK, BLK, 128], AMID, tag="qt")
                KT = pqt.tile([128, NCHUNK, BLK, 128], AMID, tag="kt")
                PDT = BF16 if (pt_bf16 and not attn_f32) else F32
                for w_m, dst in ((wq_m, QT), (wk_m, KT)):
                    for co in range(NCHUNK):
                        PT = psf.tile([128, BLK * 128], PDT, tag="f")
                        for jc in range(NCHUNK):
                            nc.tensor.matmul(
                                PT[:],
                                lhsT=w_m[:, jc, co * 128:(co + 1) * 128],
                                rhs=XT_sb[:, jc, :, :],
                                start=(jc == 0), stop=(jc == NCHUNK - 1))
                        cp_engine(qk_copy)(dst[:, co, :, :], PT[:])
                V = pv.tile([128, BLK, C], AMID, tag="v")
                for s in range(BLK):
                    VP = psf.tile([128, C], PDT, tag="f")
                    for jc in range(NCHUNK):
                        nc.tensor.matmul(
                            VP[:], lhsT=XT_sb[:, jc, s, :],
                            rhs=wv_m[:, jc, :],
                            start=(jc == 0), stop=(jc == NCHUNK - 1))
                    cp_engine(v_copy)(V[:, s, :], VP[:])
                state[b] = (QT, KT, V)

            def back_seq_std(QT, KT, V, O_sb, s, p_s, p_b, t_s="s", t_b="b",
                             blk_idx=None):
                # std orientation: S[i, j]; exp with ACT-accumulated row sums;
                # A normalized (DVE/Pool mul), A^T via PE transposes.
                S_e = p_s.tile([128, G // 2, 128], F32, tag=t_s)
                S_o = p_s.tile([128, G // 2, 128], F32, tag=t_s)
                for g in range(G):
                    p0 = 64 * (g % 2)
                    S_ps = S_e if g % 2 == 0 else S_o
                    # lhsT=Q^T, rhs=K^T  ->  S[i, j]
                    nc.tensor.matmul(
                        S_ps[:, g // 2, :],
                        lhsT=QT[p0:p0 + 64, g // 2, s, :],
                        rhs=KT[p0:p0 + 64, g // 2, s, :],
                        start=True, stop=True)
                A = pa.tile([128, G, 128], AMID, tag="a")
                ssum = pstat.tile([128, G], F32, tag="sums")
                if exp_mode == 'perhead':
                    # 8 small ACT ops; row sums accumulated for free
                    for g in range(G):
                        S_ps = S_e if g % 2 == 0 else S_o
                        nc.scalar.activation(
                            A[:, g, :], S_ps[:, g // 2, :], EXP,
                            scale=1.0 / np.sqrt(GP),
                            accum_out=ssum[:, g:g + 1])
                else:
                    # 2 big ACT ops; row sums via free-axis reduce elsewhere
                    a_ev = A[:, 0:G:2, :]
                    a_od = A[:, 1:G:2, :]
                    nc.scalar.activation(a_ev, S_e[:], EXP,
                                         scale=1.0 / np.sqrt(GP))
                    nc.scalar.activation(a_od, S_o[:], EXP,
                                         scale=1.0 / np.sqrt(GP))
                    se = ssum[:, 0:G:2].rearrange("p (g o) -> p g o", o=1)
                    so = ssum[:, 1:G:2].rearrange("p (g o) -> p g o", o=1)
                    red = (nc.gpsimd if sums_engine == 'pool'
                           else nc.vector)
                    red.reduce_sum(se, a_ev, axis=mybir.AxisListType.X)
                    red.reduce_sum(so, a_od, axis=mybir.AxisListType.X)
                rcp = pstat.tile([128, G], F32, tag="rcp")
                nc.vector.reciprocal(rcp[:], ssum[:])
                AN = pa.tile([128, G, 128], AMID, tag="an")
                rcp_b = rcp[:].rearrange("p (o g) -> p o g", o=1) \
                    .rearrange("p o g -> p g o") \
                    .broadcast_to((128, G, 128))
                if an_engine == 'pool':
                    nc.gpsimd.tensor_mul(AN[:], A[:], rcp_b)
                else:
                    nc.vector.tensor_mul(AN[:], A[:], rcp_b)
                AT = pa.tile([128, G, 128], AMID, tag="at")
                idt = id_m if not attn_f32 else id_sb
                if at_merge and not attn_f32:
                    # all 8 transposed heads into ONE bf16 psum bank (2KB),
                    # exited by a single FD-1024 DVE copy
                    ATp = p_b.tile([128, G, 128], AMID, tag=t_b)
                    for g in range(G):
                        nc.tensor.transpose(ATp[:, g, :], AN[:, g, :], idt[:])
                    cp_engine(at_copy)(AT[:], ATp[:])
                else:
                    for half in range(2):
                        ATp = p_b.tile([128, G // 2, 128], AMID, tag=t_b)
                        for gg in range(G // 2):
                            g = half * (G // 2) + gg
                            nc.tensor.transpose(ATp[:, gg, :], AN[:, g, :],
                                                idt[:])
                        cp_engine(at_copy)(
                            AT[:, half * (G // 2):(half + 1) * (G // 2), :],
                            ATp[:])
                ODT = BF16 if (ot_bf16 and not attn_f32) else F32
                OT_ps = p_b.tile([128, NCHUNK, 128], ODT, tag=t_b)
                for g in range(G):
                    p0 = 64 * (g % 2)
                    nc.tensor.matmul(
                        OT_ps[p0:p0 + 64, g // 2, :],
                        lhsT=V[:, s, 64 * g:64 * (g + 1)],
                        rhs=AT[:, g, :],
                        start=True, stop=True)
                OT_sb = pot.tile([128, NCHUNK, 128],
                                 BF16 if ODT == BF16 else F32R, tag="ot")
                cp_engine(ot_copy)(OT_sb[:], OT_ps[:])
                wo_u = wo_m if ODT == BF16 else wo_sb
                O_ps = p_b.tile([128, C], F32, tag=t_b)
                for jc in range(NCHUNK):
                    nc.tensor.matmul(
                        O_ps[:], lhsT=OT_sb[:, jc, :],
                        rhs=wo_u[:, jc, :],
                        start=(jc == 0),
                        stop=(jc == NCHUNK - 1 and not o_direct))
                if o_direct:
                    # += 1*bo via a k=1 accumulation step, then DMA the
                    # finished rows straight from PSUM to DRAM (no O_sb).
                    nc.tensor.matmul(O_ps[:], lhsT=ones1[:], rhs=bo_r[:],
                                     start=False, stop=True)
                    nc.sync.dma_start(
                        out_d[:, blk_idx * BLK + s, :], O_ps[:])
                else:
                    nc.vector.tensor_add(O_sb[:, s, :], O_ps[:], bo_sb[:])

            def back_seq_st2(QT, KT, V, O_sb, s, p_s, p_b, t_s="s", t_b="b",
                             blk_idx=None):
                # S^T-direct: exp(S^T) IS A^T; softmax sums via ones-matmul
                # column-sum broadcast; normalization folded into OT psum exit.
                # Even heads (PE row-group 0) and odd heads (row-group 1)
                # run concurrently in the array -> MUST land in different
                # PSUM banks (same-bank concurrent row-group writes hang).
                S_e = p_s.tile([128, G // 2, 128], F32, tag=t_s)
                S_o = p_s.tile([128, G // 2, 128], F32, tag=t_s)
                # evens first so exp(S_e) can start while odd S still runs
                for g in (0, 2, 4, 6, 1, 3, 5, 7):
                    p0 = 64 * (g % 2)
                    S_ps = S_e if g % 2 == 0 else S_o
                    # lhsT=K^T, rhs=Q^T  ->  S^T[j, i]
                    nc.tensor.matmul(
                        S_ps[:, g // 2, :],
                        lhsT=KT[p0:p0 + 64, g // 2, s, :],
                        rhs=QT[p0:p0 + 64, g // 2, s, :],
                        start=True, stop=True)
                AT = pa.tile([128, G, 128], AMID, tag="at")
                nc.scalar.activation(AT[:, 0:G:2, :], S_e[:], EXP,
                                     scale=1.0 / np.sqrt(GP))
                nc.scalar.activation(AT[:, 1:G:2, :], S_o[:], EXP,
                                     scale=1.0 / np.sqrt(GP))
                # rcp_t[p, c, i] = 1/colsum[g = 2c + p//64, i]
                rcp_t = pstat.tile([128, NCHUNK, 128], F32, tag="rcpt")
                if sums == 'par':
                    # Pool partition all-reduce: every partition gets all
                    # heads' column sums; no PE work.
                    s_all = pstat.tile([128, G, 128], F32, tag="sall")
                    nc.gpsimd.partition_all_reduce(
                        s_all[:], AT[:], channels=128,
                        reduce_op=bass_isa.ReduceOp.add)
                    nc.vector.reciprocal(rcp_t[0:64, :, :],
                                         s_all[0:64, 0:G:2, :])
                    nc.vector.reciprocal(rcp_t[64:128, :, :],
                                         s_all[64:128, 1:G:2, :])
                else:
                    # Column-sum broadcast: bc1[p, c*128+i] = sum_j AT[j, 2c, i]
                    # (even heads), bc2 odd; every partition gets a copy.
                    bc1 = p_b.tile([128, NCHUNK, 128], F32, tag=t_b)
                    bc2 = p_b.tile([128, NCHUNK, 128], F32, tag=t_b)
                    nc.tensor.matmul(bc1[:], lhsT=ones_m[:],
                                     rhs=AT[:, 0:G:2, :], start=True, stop=True)
                    nc.tensor.matmul(bc2[:], lhsT=ones_m[:],
                                     rhs=AT[:, 1:G:2, :], start=True, stop=True)
                    nc.vector.reciprocal(rcp_t[0:64, :, :], bc1[0:64, :, :])
                    nc.vector.reciprocal(rcp_t[64:128, :, :], bc2[64:128, :, :])
                OT_ps = p_b.tile([128, NCHUNK, 128], F32, tag=t_b)
                for g in range(G):
                    p0 = 64 * (g % 2)
                    nc.tensor.matmul(
                        OT_ps[p0:p0 + 64, g // 2, :],
                        lhsT=V[:, s, 64 * g:64 * (g + 1)],
                        rhs=AT[:, g, :],
                        start=True, stop=True)
                OT_sb = pot.tile([128, NCHUNK, 128], F32R, tag="ot")
                if ot_mul == 'dve':
                    nc.vector.tensor_mul(OT_sb[:], OT_ps[:], rcp_t[:])
                else:
                    nc.gpsimd.tensor_mul(OT_sb[:], OT_ps[:], rcp_t[:])
                O_ps = p_b.tile([128, C], F32, tag=t_b)
                for jc in range(NCHUNK):
                    nc.tensor.matmul(
                        O_ps[:], lhsT=OT_sb[:, jc, :],
                        rhs=wo_sb[:, jc, :],
                        start=(jc == 0), stop=(jc == NCHUNK - 1))
                nc.vector.tensor_add(O_sb[:, s, :], O_ps[:], bo_sb[:])

            back_seq = back_seq_std if attn_mode == 'std' else back_seq_st2

            def back(b):
                QT, KT, V = state.pop(b)
                O_sb = None if o_direct else po.tile([128, BLK, C], F32,
                                                     tag="o")
                for s in range(BLK):
                    back_seq(QT, KT, V, O_sb, s, pss, psb, blk_idx=b)
                if not o_direct:
                    nc.sync.dma_start(out_d[:, b * BLK:(b + 1) * BLK, :],
                                      O_sb[:])

            def back_pair(b1, b2):
                # Tail: interleave the last two blocks' backs at seq
                # granularity so their dependency chains overlap. The second
                # block's psum tiles borrow the front pool (idle in the tail).
                QT1, KT1, V1 = state.pop(b1)
                QT2, KT2, V2 = state.pop(b2)
                O1 = None if o_direct else po.tile([128, BLK, C], F32,
                                                    tag="o")
                O2 = None if o_direct else po.tile([128, BLK, C], F32,
                                                   tag="o")
                for s in range(BLK):
                    back_seq(QT1, KT1, V1, O1, s, pss, psb, blk_idx=b1)
                    back_seq(QT2, KT2, V2, O2, s, psf, psf, t_s="f", t_b="f",
                             blk_idx=b2)
                if not o_direct:
                    nc.sync.dma_start(out_d[:, b1 * BLK:(b1 + 1) * BLK, :],
                                      O1[:])
                    nc.sync.dma_start(out_d[:, b2 * BLK:(b2 + 1) * BLK, :],
                                      O2[:])

            def whole(X_first=None, pair_tail=True):
                # pair_tail only helps the drain of a single-shot run; in a
                # rep loop the next iteration's fronts fill the tail anyway,
                # and the borrowed front-pool psum banks would contend.
                for b in range(nblk + 1):
                    if b < nblk:
                        front(b, X_pre=(X_first if b == 0 else None))
                    if b >= 1:
                        if pair_tail and nblk >= 2 and b - 1 == nblk - 2:
                            continue  # deferred into the tail pair
                        elif pair_tail and b - 1 == nblk - 1 and nblk >= 2:
                            back_pair(nblk - 2, nblk - 1)
                        else:
                            back(b - 1)

            if reps == 1:
                whole(X_first=X0)
            else:
                with tc.For_i(0, reps, 1):
                    whole(pair_tail=False)

    nc.compile()
    return nc


def make_in_maps(x, Wq, Wk, Wv, Wo, bo, num_cores=8):
    """Full inputs -> per-core input dicts (data-parallel over N)."""
    x = np.asarray(x, dtype=np.float32)
    ident = np.eye(128, dtype=np.float32)
    wqT = np.ascontiguousarray(np.asarray(Wq, np.float32).T)
    wkT = np.ascontiguousarray(np.asarray(Wk, np.float32).T)
    wvT = np.ascontiguousarray(np.asarray(Wv, np.float32).T)
    woT = np.ascontiguousarray(np.asarray(Wo, np.float32).T)
    bo = np.asarray(bo, np.float32)
    return [{"x": np.ascontiguousarray(x[n]), "wqT": wqT, "wkT": wkT,
             "wvT": wvT, "woT": woT, "bo": bo, "ident": ident}
            for n in range(num_cores)]


_NC_CACHE = {}

# Single source of truth for the shipped configuration (test.py reads this).
KERNEL_CFG = dict(attn_mode='std', xconv='none', qk_copy='dve',
                  v_copy='dve', xt_copy='dve', an_engine='pool',
                  at_copy='dve', ot_copy='dve', exp_mode='perhead',
                  psum_bufs=(2, 3, 3),
                  sbufs={'px': 3, 'pxt': 3, 'pqt': 4, 'pv': 4, 'pa': 4,
                         'pstat': 6, 'pot': 3, 'po': 3})


def kernel(x, Wq, Wk, Wv, Wo, bo):
    import numpy as np
    from concourse import bass_utils

    if "nc" not in _NC_CACHE:
        _NC_CACHE["nc"] = build_kernel(num_cores=8, **KERNEL_CFG)
    nc = _NC_CACHE["nc"]
    in_maps = make_in_maps(x, Wq, Wk, Wv, Wo, bo, num_cores=8)
    res = bass_utils.run_bass_kernel_spmd(nc, in_maps, core_ids=list(range(8)))
    out = np.stack([res.results[c]["out"] for c in range(8)], axis=0)
    return out.astype(np.float32)


# revision 34
# speedup vs baseline: 8.2880x; 1.0162x over previous
"""Self-contained TRN2 Bass kernel for axial attention (nn_AxialAttention).

kernel(**inputs) takes FULL inputs (x [8,128,128,512], Wq/Wk/Wv/Wo [512,512],
bo [512]) and returns the FULL output [8,128,128,512] (float32).

Sharding: data-parallel over N across 8 NeuronCores (core c computes image c).

Per-core dataflow (v2), per block of BLK=4 sequences (seq = one W column,
tokens = H axis):
  X_blk  [128 t, 4 s, 512 c] f32  <- DMA
  X^T    [128 c_loc, 4 jc, 4 s, 128 t] bf16, PE transposes (f32r in, bf16 out)
  Q^T/K^T [128 co_loc, 4 co, 4 s, 128 t] bf16 = Wq/k^T.T @ X^T, psum exit on ACT
  V      [128 t, 4 s, 512 c] bf16 = X^T.T @ Wv^T, psum exit on ACT
  S^T_g  [128 j, 128 i] psum = K^T_g.T @ Q^T_g   (even/odd heads -> 2 banks)
  A^T    exp(S^T/8) directly (no transposes; softmax max-subtraction skipped:
         logits ~N(0,1), |S|<~6, exp safe in f32)
  colsum broadcast: ones^T @ A^T (per parity) -> bc psum [128, 4, 128]
  rcp_t  1/bc on the used partition halves (DVE, 2 ops)
  OT     [128 c_loc, 4 jc, 128 i] psum = V_g.T...: lhsT=V_g, rhs=A^T_g
  OT_sb  = OT * rcp_t  (softmax normalization folded into psum exit, DVE)
  O      [128 t, 512 co] = OT.T @ Wo^T (f32r) + bo (DVE psum exit)
"""
import sys
sys.path.insert(0, "/opt/trn_rl_repo")
sys.path.insert(0, "/root/.axon_site/_ro/trn_rl_repo")

import numpy as np

import concourse.bass as bass
import concourse.bacc as bacc
import concourse.tile as tile
from concourse import mybir
from concourse import bass_isa

F32 = mybir.dt.float32
F32R = mybir.dt.float32r
BF16 = mybir.dt.bfloat16
EXP = mybir.ActivationFunctionType.Exp

H = 128   # tokens per sequence (attention axis)
W = 128   # sequences per core
C = 512
G = 8     # heads
GP = C // G  # 64
BLK = 4   # sequences per block
NCHUNK = C // 128  # 4 k-chunks


def build_kernel(num_cores=8, attn_f32=False, w_total=W, reps=1,
                 qk_copy='dve', v_copy='dve', xt_copy='dve',
                 ot_mul='dve', xconv='pool', psum_bufs=(3, 2, 3), sbufs=None,
                 psum_unified=False, pre_x0=True, sums='bc', attn_mode='std',
                 an_engine='dve', at_copy='act', ot_copy='dve',
                 exp_mode='batched', sums_engine='pool',
                 pt_bf16=False, o_direct=False, ot_bf16=False,
                 at_merge=False, an_split=False, interleave=False):
    """Build + compile the Bass module. Returns nc.

    reps>1 wraps the computation in a dynamic loop (for wall-clock
    rep-differencing timing; results identical, just recomputed)."""
    nblk = w_total // BLK
    nc = bacc.Bacc("TRN2", target_bir_lowering=False, debug=False,
                   num_devices=num_cores)

    x_d = nc.dram_tensor("x", [H, w_total, C], F32R, kind="ExternalInput").ap()
    wq_d = nc.dram_tensor("wqT", [C, C], F32R, kind="ExternalInput").ap()
    wk_d = nc.dram_tensor("wkT", [C, C], F32R, kind="ExternalInput").ap()
    wv_d = nc.dram_tensor("wvT", [C, C], F32R, kind="ExternalInput").ap()
    wo_d = nc.dram_tensor("woT", [C, C], F32R, kind="ExternalInput").ap()
    bo_d = nc.dram_tensor("bo", [C], F32, kind="ExternalInput").ap()
    id_d = nc.dram_tensor("ident", [128, 128], F32R, kind="ExternalInput").ap()
    out_d = nc.dram_tensor("out", [H, w_total, C], F32, kind="ExternalOutput").ap()

    AMID = F32R if attn_f32 else BF16

    sb_bufs = sbufs or {}
    def B(name, d):
        return sb_bufs.get(name, d)

    def cp_engine(which):
        return nc.scalar.copy if which == 'act' else nc.vector.tensor_copy

    with tile.TileContext(nc) as tc:
        with tc.tile_pool(name="consts", bufs=1) as consts, \
             tc.tile_pool(name="px", bufs=B('px', 3)) as px, \
             tc.tile_pool(name="pxt", bufs=B('pxt', 3)) as pxt, \
             tc.tile_pool(name="pqt", bufs=B('pqt', 3)) as pqt, \
             tc.tile_pool(name="pv", bufs=B('pv', 3)) as pv, \
             tc.tile_pool(name="pa", bufs=B('pa', 4)) as pa, \
             tc.tile_pool(name="pstat", bufs=B('pstat', 4)) as pstat, \
             tc.tile_pool(name="pot", bufs=B('pot', 3)) as pot, \
             tc.tile_pool(name="po", bufs=B('po', 3)) as po, \
             tc.tile_pool(name="psf", bufs=(8 if psum_unified else psum_bufs[0]),
                          space="PSUM") as psf, \
             tc.tile_pool(name="pss", bufs=psum_bufs[1], space="PSUM") as _pss, \
             tc.tile_pool(name="psb", bufs=psum_bufs[2], space="PSUM") as _psb:
            if psum_unified:
                class _U:
                    _n = [0]
                    def tile(self, shape, dtype, tag=None):
                        self._n[0] += 1
                        return psf.tile(shape, dtype, tag="f",
                                        name=f"u{self._n[0]}")
                pss = psb = _U()
            else:
                pss, psb = _pss, _psb

            # ---- block-0 input DMA first: the first PE work (transposes of
            # X0) must not wait behind 4MB of weight DMAs in the queue ----
            X0 = None
            if pre_x0 and reps == 1:
                X0 = px.tile([128, BLK, C], F32R, tag="x")
                nc.sync.dma_start(X0[:], x_d[:, 0:BLK, :])

            # ---- constants (id first: the first PE work -- transposes of X0
            # -- needs only X0 + id, not the 4MB of weights) ----
            id_sb = consts.tile([128, 128], F32R, tag="id")
            nc.sync.dma_start(id_sb[:], id_d[:])
            wq_sb = consts.tile([128, NCHUNK, C], F32R, tag="wq")
            wk_sb = consts.tile([128, NCHUNK, C], F32R, tag="wk")
            wv_sb = consts.tile([128, NCHUNK, C], F32R, tag="wv")
            wo_sb = consts.tile([128, NCHUNK, C], F32R, tag="wo")
            for w_sb, w_d in ((wq_sb, wq_d), (wk_sb, wk_d), (wv_sb, wv_d),
                              (wo_sb, wo_d)):
                nc.sync.dma_start(w_sb[:], w_d.rearrange("(j p) c -> p j c", p=128))
            bo_sb = consts.tile([128, C], F32, tag="bo")
            nc.sync.dma_start(
                bo_sb[:],
                bo_d.rearrange("(o c) -> o c", o=1).broadcast_to((128, C)))
            ones_m = consts.tile([128, 128], AMID, tag="ones")
            nc.vector.memset(ones_m[:], 1.0)
            if o_direct:
                ones1 = consts.tile([1, 128], F32R, tag="ones1")
                nc.vector.memset(ones1[:], 1.0)
                bo_r = consts.tile([1, C], F32R, tag="bor")
                nc.sync.dma_start(bo_r[:].bitcast(F32),
                                  bo_d.rearrange("(o c) -> o c", o=1))
            if attn_f32:
                id_m = id_sb
            else:
                id_m = consts.tile([128, 128], BF16, tag="idm")
                nc.vector.tensor_copy(id_m[:], id_sb[:].bitcast(F32))
            if attn_f32:
                wq_m, wk_m, wv_m = wq_sb, wk_sb, wv_sb
            else:
                wq_m = consts.tile([128, NCHUNK, C], BF16, tag="wqm")
                wk_m = consts.tile([128, NCHUNK, C], BF16, tag="wkm")
                wv_m = consts.tile([128, NCHUNK, C], BF16, tag="wvm")
                for dst, src in ((wq_m, wq_sb), (wk_m, wk_sb), (wv_m, wv_sb)):
                    nc.vector.tensor_copy(dst[:], src[:].bitcast(F32))
            wo_m = wo_sb
            if ot_bf16 and not attn_f32:
                wo_m = consts.tile([128, NCHUNK, C], BF16, tag="wom")
                nc.vector.tensor_copy(wo_m[:], wo_sb[:].bitcast(F32))

            state = {}

            def front(b, X_pre=None):
                front_a(b, X_pre)
                front_b(b)

            def front_a(b, X_pre=None):
                if X_pre is not None:
                    X_blk = X_pre
                else:
                    X_blk = px.tile([128, BLK, C], F32R, tag="x")
                    nc.sync.dma_start(X_blk[:], x_d[:, b * BLK:(b + 1) * BLK, :])
                if attn_f32 or xconv == 'none':
                    X_m, id_t, TDT = X_blk, id_sb, F32R
                else:
                    X_m = px.tile([128, BLK, C], BF16, tag="xm")
                    conv = (nc.gpsimd.tensor_copy if xconv == 'pool'
                            else nc.vector.tensor_copy)
                    conv(X_m[:], X_blk[:].bitcast(F32))
                    id_t, TDT = id_m, BF16
                XT_sb = pxt.tile([128, NCHUNK, BLK, 128], AMID, tag="xt")
                for s in range(BLK):
                    XT_ps = psf.tile([128, NCHUNK, 128], TDT, tag="f")
                    for jc in range(NCHUNK):
                        nc.tensor.transpose(
                            XT_ps[:, jc, :],
                            X_m[:, s, jc * 128:(jc + 1) * 128], id_t[:])
                    cp_engine(xt_copy)(XT_sb[:, :, s, :], XT_ps[:])
                QT = pqt.tile([128, NCHUNK, BLK, 128], AMID, tag="qt")
                KT = pqt.tile([128, NCHUNK, BLK, 128], AMID, tag="kt")
                PDT = BF16 if (pt_bf16 and not attn_f32) else F32
                for w_m, dst in ((wq_m, QT), (wk_m, KT)):
                    for co in range(NCHUNK):
                        PT = psf.tile([128, BLK * 128], PDT, tag="f")
                        for jc in range(NCHUNK):
                            nc.tensor.matmul(
                                PT[:],
                                lhsT=w_m[:, jc, co * 128:(co + 1) * 128],
                                rhs=XT_sb[:, jc, :, :],
                                start=(jc == 0), stop=(jc == NCHUNK - 1))
                        cp_engine(qk_copy)(dst[:, co, :, :], PT[:])
                state[b] = [QT, KT, None, XT_sb, PDT]

            def front_b(b):
                QT, KT, _, XT_sb, PDT = state[b]
                V = pv.tile([128, BLK, C], AMID, tag="v")
                for s in range(BLK):
                    VP = psf.tile([128, C], PDT, tag="f")
                    for jc in range(NCHUNK):
                        nc.tensor.matmul(
                            VP[:], lhsT=XT_sb[:, jc, s, :],
                            rhs=wv_m[:, jc, :],
                            start=(jc == 0), stop=(jc == NCHUNK - 1))
                    cp_engine(v_copy)(V[:, s, :], VP[:])
                state[b] = (QT, KT, V)

            def back_seq_std(QT, KT, V, O_sb, s, p_s, p_b, t_s="s", t_b="b",
                             blk_idx=None):
                # std orientation: S[i, j]; exp with ACT-accumulated row sums;
                # A normalized (DVE/Pool mul), A^T via PE transposes.
                S_e = p_s.tile([128, G // 2, 128], F32, tag=t_s)
                S_o = p_s.tile([128, G // 2, 128], F32, tag=t_s)
                s_order = (0, 2, 4, 6, 1, 3, 5, 7) if an_split else range(G)
                for g in s_order:
                    p0 = 64 * (g % 2)
                    S_ps = S_e if g % 2 == 0 else S_o
                    # lhsT=Q^T, rhs=K^T  ->  S[i, j]
                    nc.tensor.matmul(
                        S_ps[:, g // 2, :],
                        lhsT=QT[p0:p0 + 64, g // 2, s, :],
                        rhs=KT[p0:p0 + 64, g // 2, s, :],
                        start=True, stop=True)
                A = pa.tile([128, G, 128], AMID, tag="a")
                ssum = pstat.tile([128, G], F32, tag="sums")
                if an_split:
                    # per-parity pipeline: even heads' exp/rcp/normalize/
                    # transpose overlap the odd heads' exps
                    AN = pa.tile([128, G, 128], AMID, tag="an")
                    AT = pa.tile([128, G, 128], AMID, tag="at")
                    idt = id_m if not attn_f32 else id_sb
                    for par, S_ps in ((0, S_e), (1, S_o)):
                        for g in range(par, G, 2):
                            nc.scalar.activation(
                                A[:, g, :], S_ps[:, g // 2, :], EXP,
                                scale=1.0 / np.sqrt(GP),
                                accum_out=ssum[:, g:g + 1])
                        rcp_h = pstat.tile([128, G // 2], F32, tag="rcph")
                        nc.vector.reciprocal(rcp_h[:], ssum[:, par:G:2])
                        rb = rcp_h[:].rearrange("p (o g) -> p o g", o=1)                             .rearrange("p o g -> p g o")                             .broadcast_to((128, G // 2, 128))
                        anh = AN[:, par:G:2, :]
                        if an_engine == 'pool':
                            nc.gpsimd.tensor_mul(anh, A[:, par:G:2, :], rb)
                        else:
                            nc.vector.tensor_mul(anh, A[:, par:G:2, :], rb)
                        ATp = p_b.tile([128, G // 2, 128], AMID, tag=t_b)
                        for gg in range(G // 2):
                            nc.tensor.transpose(ATp[:, gg, :],
                                                AN[:, 2 * gg + par, :], idt[:])
                        cp_engine(at_copy)(AT[:, par:G:2, :], ATp[:])
                elif exp_mode == 'perhead':
                    # 8 small ACT ops; row sums accumulated for free
                    for g in range(G):
                        S_ps = S_e if g % 2 == 0 else S_o
                        nc.scalar.activation(
                            A[:, g, :], S_ps[:, g // 2, :], EXP,
                            scale=1.0 / np.sqrt(GP),
                            accum_out=ssum[:, g:g + 1])
                else:
                    # 2 big ACT ops; row sums via free-axis reduce elsewhere
                    a_ev = A[:, 0:G:2, :]
                    a_od = A[:, 1:G:2, :]
                    nc.scalar.activation(a_ev, S_e[:], EXP,
                                         scale=1.0 / np.sqrt(GP))
                    nc.scalar.activation(a_od, S_o[:], EXP,
                                         scale=1.0 / np.sqrt(GP))
                    se = ssum[:, 0:G:2].rearrange("p (g o) -> p g o", o=1)
                    so = ssum[:, 1:G:2].rearrange("p (g o) -> p g o", o=1)
                    red = (nc.gpsimd if sums_engine == 'pool'
                           else nc.vector)
                    red.reduce_sum(se, a_ev, axis=mybir.AxisListType.X)
                    red.reduce_sum(so, a_od, axis=mybir.AxisListType.X)
                if not an_split:
                    rcp = pstat.tile([128, G], F32, tag="rcp")
                    nc.vector.reciprocal(rcp[:], ssum[:])
                    AN = pa.tile([128, G, 128], AMID, tag="an")
                    rcp_b = rcp[:].rearrange("p (o g) -> p o g", o=1) \
                        .rearrange("p o g -> p g o") \
                        .broadcast_to((128, G, 128))
                    if an_engine == 'pool':
                        nc.gpsimd.tensor_mul(AN[:], A[:], rcp_b)
                    else:
                        nc.vector.tensor_mul(AN[:], A[:], rcp_b)
                    AT = pa.tile([128, G, 128], AMID, tag="at")
                    idt = id_m if not attn_f32 else id_sb
                    if at_merge and not attn_f32:
                        # all 8 heads into ONE bf16 psum bank (2KB), exited
                        # by a single FD-1024 DVE copy
                        ATp = p_b.tile([128, G, 128], AMID, tag=t_b)
                        for g in range(G):
                            nc.tensor.transpose(ATp[:, g, :], AN[:, g, :],
                                                idt[:])
                        cp_engine(at_copy)(AT[:], ATp[:])
                    else:
                        for half in range(2):
                            ATp = p_b.tile([128, G // 2, 128], AMID, tag=t_b)
                            for gg in range(G // 2):
                                g = half * (G // 2) + gg
                                nc.tensor.transpose(ATp[:, gg, :],
                                                    AN[:, g, :], idt[:])
                            cp_engine(at_copy)(
                                AT[:, half * (G // 2):(half + 1) * (G // 2),
                                   :],
                                ATp[:])
                ODT = BF16 if (ot_bf16 and not attn_f32) else F32
                OT_ps = p_b.tile([128, NCHUNK, 128], ODT, tag=t_b)
                for g in range(G):
                    p0 = 64 * (g % 2)
                    nc.tensor.matmul(
                        OT_ps[p0:p0 + 64, g // 2, :],
                        lhsT=V[:, s, 64 * g:64 * (g + 1)],
                        rhs=AT[:, g, :],
                        start=True, stop=True)
                OT_sb = pot.tile([128, NCHUNK, 128],
                                 BF16 if ODT == BF16 else F32R, tag="ot")
                cp_engine(ot_copy)(OT_sb[:], OT_ps[:])
                wo_u = wo_m if ODT == BF16 else wo_sb
                O_ps = p_b.tile([128, C], F32, tag=t_b)
                for jc in range(NCHUNK):
                    nc.tensor.matmul(
                        O_ps[:], lhsT=OT_sb[:, jc, :],
                        rhs=wo_u[:, jc, :],
                        start=(jc == 0),
                        stop=(jc == NCHUNK - 1 and not o_direct))
                if o_direct:
                    # += 1*bo via a k=1 accumulation step, then DMA the
                    # finished rows straight from PSUM to DRAM (no O_sb).
                    nc.tensor.matmul(O_ps[:], lhsT=ones1[:], rhs=bo_r[:],
                                     start=False, stop=True)
                    nc.sync.dma_start(
                        out_d[:, blk_idx * BLK + s, :], O_ps[:])
                else:
                    nc.vector.tensor_add(O_sb[:, s, :], O_ps[:], bo_sb[:])

            def back_seq_st2(QT, KT, V, O_sb, s, p_s, p_b, t_s="s", t_b="b",
                             blk_idx=None):
                # S^T-direct: exp(S^T) IS A^T; softmax sums via ones-matmul
                # column-sum broadcast; normalization folded into OT psum exit.
                # Even heads (PE row-group 0) and odd heads (row-group 1)
                # run concurrently in the array -> MUST land in different
                # PSUM banks (same-bank concurrent row-group writes hang).
                S_e = p_s.tile([128, G // 2, 128], F32, tag=t_s)
                S_o = p_s.tile([128, G // 2, 128], F32, tag=t_s)
                # evens first so exp(S_e) can start while odd S still runs
                for g in (0, 2, 4, 6, 1, 3, 5, 7):
                    p0 = 64 * (g % 2)
                    S_ps = S_e if g % 2 == 0 else S_o
                    # lhsT=K^T, rhs=Q^T  ->  S^T[j, i]
                    nc.tensor.matmul(
                        S_ps[:, g // 2, :],
                        lhsT=KT[p0:p0 + 64, g // 2, s, :],
                        rhs=QT[p0:p0 + 64, g // 2, s, :],
                        start=True, stop=True)
                AT = pa.tile([128, G, 128], AMID, tag="at")
                nc.scalar.activation(AT[:, 0:G:2, :], S_e[:], EXP,
                                     scale=1.0 / np.sqrt(GP))
                nc.scalar.activation(AT[:, 1:G:2, :], S_o[:], EXP,
                                     scale=1.0 / np.sqrt(GP))
                # rcp_t[p, c, i] = 1/colsum[g = 2c + p//64, i]
                rcp_t = pstat.tile([128, NCHUNK, 128], F32, tag="rcpt")
                if sums == 'par':
                    # Pool partition all-reduce: every partition gets all
                    # heads' column sums; no PE work.
                    s_all = pstat.tile([128, G, 128], F32, tag="sall")
                    nc.gpsimd.partition_all_reduce(
                        s_all[:], AT[:], channels=128,
                        reduce_op=bass_isa.ReduceOp.add)
                    nc.vector.reciprocal(rcp_t[0:64, :, :],
                                         s_all[0:64, 0:G:2, :])
                    nc.vector.reciprocal(rcp_t[64:128, :, :],
                                         s_all[64:128, 1:G:2, :])
                else:
                    # Column-sum broadcast: bc1[p, c*128+i] = sum_j AT[j, 2c, i]
                    # (even heads), bc2 odd; every partition gets a copy.
                    bc1 = p_b.tile([128, NCHUNK, 128], F32, tag=t_b)
                    bc2 = p_b.tile([128, NCHUNK, 128], F32, tag=t_b)
                    nc.tensor.matmul(bc1[:], lhsT=ones_m[:],
                                     rhs=AT[:, 0:G:2, :], start=True, stop=True)
                    nc.tensor.matmul(bc2[:], lhsT=ones_m[:],
                                     rhs=AT[:, 1:G:2, :], start=True, stop=True)
                    nc.vector.reciprocal(rcp_t[0:64, :, :], bc1[0:64, :, :])
                    nc.vector.reciprocal(rcp_t[64:128, :, :], bc2[64:128, :, :])
                OT_ps = p_b.tile([128, NCHUNK, 128], F32, tag=t_b)
                for g in range(G):
                    p0 = 64 * (g % 2)
                    nc.tensor.matmul(
                        OT_ps[p0:p0 + 64, g // 2, :],
                        lhsT=V[:, s, 64 * g:64 * (g + 1)],
                        rhs=AT[:, g, :],
                        start=True, stop=True)
                OT_sb = pot.tile([128, NCHUNK, 128], F32R, tag="ot")
                if ot_mul == 'dve':
                    nc.vector.tensor_mul(OT_sb[:], OT_ps[:], rcp_t[:])
                else:
                    nc.gpsimd.tensor_mul(OT_sb[:], OT_ps[:], rcp_t[:])
                O_ps = p_b.tile([128, C], F32, tag=t_b)
                for jc in range(NCHUNK):
                    nc.tensor.matmul(
                        O_ps[:], lhsT=OT_sb[:, jc, :],
                        rhs=wo_sb[:, jc, :],
                        start=(jc == 0), stop=(jc == NCHUNK - 1))
                nc.vector.tensor_add(O_sb[:, s, :], O_ps[:], bo_sb[:])

            back_seq = back_seq_std if attn_mode == 'std' else back_seq_st2

            def back(b):
                QT, KT, V = state.pop(b)
                O_sb = None if o_direct else po.tile([128, BLK, C], F32,
                                                     tag="o")
                for s in range(BLK):
                    back_seq(QT, KT, V, O_sb, s, pss, psb, blk_idx=b)
                if not o_direct:
                    nc.sync.dma_start(out_d[:, b * BLK:(b + 1) * BLK, :],
                                      O_sb[:])

            def back_pair(b1, b2):
                # Tail: interleave the last two blocks' backs at seq
                # granularity so their dependency chains overlap. The second
                # block's psum tiles borrow the front pool (idle in the tail).
                QT1, KT1, V1 = state.pop(b1)
                QT2, KT2, V2 = state.pop(b2)
                O1 = None if o_direct else po.tile([128, BLK, C], F32,
                                                    tag="o")
                O2 = None if o_direct else po.tile([128, BLK, C], F32,
                                                   tag="o")
                for s in range(BLK):
                    back_seq(QT1, KT1, V1, O1, s, pss, psb, blk_idx=b1)
                    back_seq(QT2, KT2, V2, O2, s, psf, psf, t_s="f", t_b="f",
                             blk_idx=b2)
                if not o_direct:
                    nc.sync.dma_start(out_d[:, b1 * BLK:(b1 + 1) * BLK, :],
                                      O1[:])
                    nc.sync.dma_start(out_d[:, b2 * BLK:(b2 + 1) * BLK, :],
                                      O2[:])

            def whole(X_first=None, pair_tail=True):
                # pair_tail only helps the drain of a single-shot run; in a
                # rep loop the next iteration's fronts fill the tail anyway,
                # and the borrowed front-pool psum banks would contend.
                for b in range(nblk + 1):
                    if b < nblk:
                        front(b, X_pre=(X_first if b == 0 else None))
                    if b >= 1:
                        if pair_tail and nblk >= 2 and b - 1 == nblk - 2:
                            continue  # deferred into the tail pair
                        elif pair_tail and b - 1 == nblk - 1 and nblk >= 2:
                            back_pair(nblk - 2, nblk - 1)
                        else:
                            back(b - 1)

            def whole_il(X_first=None):
                # software-pipelined emission: back(b-1) seqs alternate with
                # front(b) stages so each engine's in-order queue always has
                # an independent stream to fall through to.
                front(0, X_pre=X_first)
                for b in range(1, nblk + 1):
                    bb = b - 1
                    QT, KT, V = state.pop(bb)
                    O_sb = po.tile([128, BLK, C], F32, tag="o")
                    back_seq(QT, KT, V, O_sb, 0, pss, psb, blk_idx=bb)
                    if b < nblk:
                        front_a(b)
                    back_seq(QT, KT, V, O_sb, 1, pss, psb, blk_idx=bb)
                    back_seq(QT, KT, V, O_sb, 2, pss, psb, blk_idx=bb)
                    if b < nblk:
                        front_b(b)
                    back_seq(QT, KT, V, O_sb, 3, pss, psb, blk_idx=bb)
                    nc.sync.dma_start(out_d[:, bb * BLK:(bb + 1) * BLK, :],
                                      O_sb[:])

            w_fn = whole_il if interleave else whole
            if reps == 1:
                if interleave:
                    whole_il(X_first=X0)
                else:
                    whole(X_first=X0)
            else:
                with tc.For_i(0, reps, 1):
                    if interleave:
                        whole_il()
                    else:
                        whole(pair_tail=False)

    nc.compile()
    return nc


def make_in_maps(x, Wq, Wk, Wv, Wo, bo, num_cores=8):
    """Full inputs -> per-core input dicts (data-parallel over N)."""
    x = np.asarray(x, dtype=np.float32)
    ident = np.eye(128, dtype=np.float32)
    wqT = np.ascontiguousarray(np.asarray(Wq, np.float32).T)
    wkT = np.ascontiguousarray(np.asarray(Wk, np.float32).T)
    wvT = np.ascontiguousarray(np.asarray(Wv, np.float32).T)
    woT = np.ascontiguousarray(np.asarray(Wo, np.float32).T)
    bo = np.asarray(bo, np.float32)
    return [{"x": np.ascontiguousarray(x[n]), "wqT": wqT, "wkT": wkT,
             "wvT": wvT, "woT": woT, "bo": bo, "ident": ident}
            for n in range(num_cores)]


_NC_CACHE = {}

# Single source of truth for the shipped configuration (test.py reads this).
KERNEL_CFG = dict(attn_mode='std', xconv='none', qk_copy='dve',
                  v_copy='dve', xt_copy='dve', an_engine='pool',
                  at_copy='dve', ot_copy='dve', exp_mode='perhead',
                  psum_bufs=(2, 3, 3),
                  sbufs={'px': 3, 'pxt': 3, 'pqt': 4, 'pv': 4, 'pa': 4,
                         'pstat': 6, 'pot': 3, 'po': 3})


def kernel(x, Wq, Wk, Wv, Wo, bo):
    import numpy as np
    from concourse import bass_utils

    if "nc" not in _NC_CACHE:
        _NC_CACHE["nc"] = build_kernel(num_cores=8, **KERNEL_CFG)
    nc = _NC_CACHE["nc"]
    in_maps = make_in_maps(x, Wq, Wk, Wv, Wo, bo, num_cores=8)
    res = bass_utils.run_bass_kernel_spmd(nc, in_maps, core_ids=list(range(8)))
    out = np.stack([res.results[c]["out"] for c in range(8)], axis=0)
    return out.astype(np.float32)
